# revision 17
# baseline (speedup 1.0000x reference)
"""GCN message-passing kernel for 8 TRN2 NeuronCores (Bass/Tile), v3.

Math (equivalent to the PyG-style reference):
    deg[i]  = 1 + #{edges with target i}              (self-loops added)
    dinv    = deg^-1/2
    y[i]    = dinv[i] * sum_{j -> i} dinv[j] * x[j]   (incl. self loop j=i)
    g       = relu(y @ Wg^T + bg)
    h       = relu(g @ W1^T + b1)
    out     = sigmoid(relu(h @ W2^T + b2))

Design notes:
  - Host does index work only: shard/sort/pad edges, count degrees.
    Device computes dinv = rsqrt(deg), the scaled table xs = dinv*x (fp8),
    all matmuls and activations.
  - Nodes sharded: core k owns 98 blocks of 128 targets, grouped into 14
    grps of 7 (one PSUM bank per block accumulator).
  - Edges sorted by (grp, src sub-table, target block); each (grp, sub,
    block) segment is padded to a cross-core common, 128-multiple length,
    so one SPMD program serves all cores and every 128-edge tile maps to
    exactly one target block.
  - dma_gather pulls xs source rows in 1024-idx calls cycled over 4 SWDGE
    queues (4x descriptor-gen parallelism; single-queue is Q7-bound at
    ~116 idx/us, 4 queues reach ~360 idx/us).
  - Aggregation: per tile, a pure one-hot fp8 matrix (one DVE is_equal
    from a colrel table) scatters gathered rows into the block's PSUM via
    TensorE matmul; dinv[target] is applied by the ACT drain (Copy*scale).
  - Fused MLP tail per grp (deferred one grp for overlap): X-bar DMA
    transposes, 2x256 matmuls, relu/sigmoid on ACT.
"""

import math

import numpy as np
import ml_dtypes

P = 128
C = 256
CO = 2                      # C // P
NCORE = 8
N = 100000
NB = 98                     # blocks per core
NBLK = NB * NCORE           # 784
NPAD = NBLK * P             # 100352
GRP = 7                     # blocks per psum group
NGRP = NB // GRP            # 14
SUB = 4                     # gather sub-tables (int16 index limit)
SUBROWS = NPAD // SUB       # 25088
SUBBLK = SUBROWS // P       # 196
NGQ = NGRP * SUB            # groups per core
GMAX = 8                    # tiles per dma_gather call (1024-idx ring limit)
NQ = 4                      # SWDGE queues
BCO = 16                    # coeff tiles built per DVE instruction
XB = 7                      # x blocks per xs-build step

_BF16 = ml_dtypes.bfloat16
_FP8 = ml_dtypes.float8_e4m3

LAST_EXEC_NS = None


# ----------------------------------------------------------------------------
# host-side preprocessing (index work: shard, sort, pad, count)
# ----------------------------------------------------------------------------

def _preprocess(edge_index):
    row = np.ascontiguousarray(edge_index[0]).astype(np.int64)
    col = np.ascontiguousarray(edge_index[1]).astype(np.int64)
    loop = np.arange(N, dtype=np.int64)
    row = np.concatenate([row, loop])
    col = np.concatenate([col, loop])

    deg = np.bincount(col, minlength=NPAD).astype(np.float32)
    deg[N:] = 1.0
    deg_tbl = np.ascontiguousarray(deg.reshape(NBLK, P).T)      # [P, NBLK]

    blk = col >> 7
    core = blk // NB
    bl = blk - core * NB
    g = bl // GRP
    s = bl % GRP
    q = row // SUBROWS
    gq = g * SUB + q

    order = np.lexsort((col, gq, core))
    row_s = row[order]
    col_s = col[order]
    core_s = core[order]
    q_s = q[order]

    # segment = (core, gq, slot); common padded length per (gq, slot),
    # NOT tile-aligned (tiles straddling a slot boundary get two entries)
    gqs = (gq * GRP + s)
    seg = core * (NGQ * GRP) + gqs
    seg_s = seg[order]
    cnt3 = np.bincount(seg, minlength=NCORE * NGQ * GRP).reshape(NCORE, -1)
    S = cnt3.max(axis=0).reshape(NGQ, GRP)                      # [NGQ, GRP]
    soff = np.zeros((NGQ, GRP + 1), np.int64)
    np.cumsum(S, axis=1, out=soff[:, 1:])
    L = soff[:, -1]                                             # edges per gq
    T = (L + P - 1) // P                                        # tiles per gq
    toff = np.zeros(NGQ + 1, np.int64)
    np.cumsum(T, out=toff[1:])
    NTILE = int(toff[-1])
    NIDX = NTILE * P

    # entry schedule: (gq, tile, slots overlapping) in order; BCO-pad per grp
    Tmax = int(T.max())
    lut = np.full((NGQ, Tmax, GRP), -1, np.int64)
    e_gq, e_t, e_s = [], [], []
    e_start, e_stop = [], []
    seen_start = set()
    grp_nmm = []
    m = 0
    for gg in range(NGRP):
        m0 = m
        for qq in range(SUB):
            gqi = gg * SUB + qq
            t_lo = soff[gqi, :-1] // P
            t_hi = (soff[gqi, 1:] - 1) // P + 1
            for t in range(int(T[gqi])):
                for ss in range(GRP):
                    if t_lo[ss] <= t < t_hi[ss]:
                        lut[gqi, t, ss] = m
                        e_gq.append(gqi); e_t.append(t); e_s.append(ss)
                        key = (gg, ss)
                        e_start.append(key not in seen_start)
                        seen_start.add(key)
                        e_stop.append(False)
                        m += 1
        seen = set()
        for i in range(m - 1, m0 - 1, -1):
            if e_s[i] not in seen:
                e_stop[i] = True
                seen.add(e_s[i])
                if len(seen) == GRP:
                    break
        npad = (-(m - m0)) % BCO
        for _ in range(npad):
            e_gq.append(-1); e_t.append(0); e_s.append(0)
            e_start.append(False); e_stop.append(False)
        m += npad
        grp_nmm.append((m0, m - m0))
    NMM = m
    e_gq = np.array(e_gq); e_t = np.array(e_t); e_s = np.array(e_s)
    e_start = np.array(e_start); e_stop = np.array(e_stop)

    # per-core data tables
    idx_dat = np.zeros((NCORE, NIDX), np.int16)
    colrel_dat = np.full((NCORE, NMM * P), 254.0, np.float32)

    seg_start = np.zeros(NCORE * NGQ * GRP + 1, np.int64)
    np.cumsum(cnt3.reshape(-1), out=seg_start[1:])
    pos_in_seg = np.arange(len(row_s), dtype=np.int64) - seg_start[seg_s]
    gqs_s = gqs[order]
    pos_in_gq = soff[:, :-1].reshape(-1)[gqs_s] + pos_in_seg    # within gq
    tile_e = pos_in_gq // P
    p_e = pos_in_gq % P
    gpos = toff[gqs_s // GRP] * P + pos_in_gq
    m_e = lut[gqs_s // GRP, tile_e, gqs_s % GRP]
    assert (m_e >= 0).all()
    idx_val = (row_s - q_s * SUBROWS).astype(np.int16)
    colv = (col_s & 127).astype(np.float32)
    flat_m = m_e * P + p_e
    for k in range(NCORE):
        sel = core_s == k
        idx_dat[k, gpos[sel]] = idx_val[sel]
        colrel_dat[k, flat_m[sel]] = colv[sel]

    idx_in = np.ascontiguousarray(
        np.tile(idx_dat.reshape(NCORE, NIDX // 16, 16).transpose(0, 2, 1),
                (1, 8, 1)))                                 # [NCORE,128,NIDX//16]
    colrel_in = np.ascontiguousarray(
        colrel_dat.reshape(NCORE, NMM, P).transpose(0, 2, 1)).astype(_BF16)

    deg_own = np.stack([deg_tbl[:, k * NB:(k + 1) * NB] for k in range(NCORE)])

    sched = dict(T=T, toff=toff, NTILE=NTILE, NMM=NMM, grp_nmm=grp_nmm,
                 e_gq=e_gq, e_t=e_t, e_s=e_s, e_start=e_start, e_stop=e_stop)
    return sched, idx_in, colrel_in, deg_tbl, np.ascontiguousarray(deg_own)


def _prep_weights(W_gcn, b_gcn, W1, b1, W2, b2):
    def wT(W):  # [C,C] -> lhsT layout [128, CO, C]: [p, ci, o] = W[o, ci*128+p]
        return np.ascontiguousarray(
            np.asarray(W).T.reshape(CO, P, C).transpose(1, 0, 2)).astype(_BF16)
    w2col = np.ascontiguousarray(
        np.asarray(W2).reshape(C).reshape(CO, P).transpose(1, 0)[:, :, None]
    ).astype(_BF16)
    bg = np.ascontiguousarray(np.asarray(b_gcn).reshape(CO, P).T).astype(np.float32)
    bb1 = np.ascontiguousarray(np.asarray(b1).reshape(CO, P).T).astype(np.float32)
    iota16 = np.tile(np.arange(P, dtype=np.float32), (P, BCO)).astype(_BF16)
    return dict(wgcnT=wT(W_gcn), w1T=wT(W1), w2col=w2col, bgcn=bg, b1=bb1,
                b2=float(np.asarray(b2).reshape(-1)[0]), iota16=iota16)


# ----------------------------------------------------------------------------
# device program (SPMD: one program, 8 cores; per-core data differs)
# ----------------------------------------------------------------------------

def _build(sched):
    from concourse import bacc, mybir
    from concourse import tile as ctile

    T = sched["T"]
    toff = sched["toff"]
    NTILE = sched["NTILE"]
    NMM = sched["NMM"]
    grp_nmm = sched["grp_nmm"]
    e_gq = sched["e_gq"]
    e_t = sched["e_t"]
    e_s = sched["e_s"]
    e_start = sched["e_start"]
    e_stop = sched["e_stop"]
    TGQMAX = int(T.max())

    f32 = mybir.dt.float32
    bf16 = mybir.dt.bfloat16
    fp8 = mybir.dt.float8e4
    i16 = mybir.dt.int16
    AF = mybir.ActivationFunctionType
    OP = mybir.AluOpType

    nc = bacc.Bacc(None, target_bir_lowering=False, debug=False,
                   num_devices=NCORE, num_swdge_queues=NQ)

    x_in = nc.dram_tensor("xb", [NPAD, C], fp8, kind="ExternalInput")
    idx_in = nc.dram_tensor("idx", [P, NTILE * 8], i16, kind="ExternalInput")
    colrel_in = nc.dram_tensor("colrel", [P, NMM], bf16, kind="ExternalInput")
    deg_in = nc.dram_tensor("dega", [P, NBLK], f32, kind="ExternalInput")
    dego_in = nc.dram_tensor("dego", [P, NB], f32, kind="ExternalInput")
    wgcnT_in = nc.dram_tensor("wgcnT", [P, CO, C], bf16, kind="ExternalInput")
    w1T_in = nc.dram_tensor("w1T", [P, CO, C], bf16, kind="ExternalInput")
    w2col_in = nc.dram_tensor("w2col", [P, CO, 1], bf16, kind="ExternalInput")
    bgcn_in = nc.dram_tensor("bgcn", [P, CO], f32, kind="ExternalInput")
    b1_in = nc.dram_tensor("b1", [P, CO], f32, kind="ExternalInput")
    b2_in = nc.dram_tensor("b2t", [P, 1], f32, kind="ExternalInput")
    iota_in = nc.dram_tensor("iota16", [P, BCO * P], bf16, kind="ExternalInput")

    z_out = nc.dram_tensor("z", [P, NB], f32, kind="ExternalOutput")
    xs_q = [nc.dram_tensor(f"xs{q}", [SUBROWS, C], fp8) for q in range(SUB)]

    with ctile.TileContext(nc) as tc:
        with tc.tile_pool(name="const", bufs=1) as CPool:
            colrel_sb = CPool.tile([P, NMM], bf16)
            nc.sync.dma_start(colrel_sb[:], colrel_in[:])
            iota_sb = CPool.tile([P, BCO, P], bf16)
            nc.sync.dma_start(iota_sb[:],
                              iota_in[:].rearrange("p (j f) -> p j f", f=P))
            wgcnT_sb = CPool.tile([P, CO, C], bf16)
            nc.sync.dma_start(wgcnT_sb[:], wgcnT_in[:])
            w1T_sb = CPool.tile([P, CO, C], bf16)
            nc.sync.dma_start(w1T_sb[:], w1T_in[:])
            w2col_sb = CPool.tile([P, CO, 1], bf16)
            nc.sync.dma_start(w2col_sb[:], w2col_in[:])
            bgcn_sb = CPool.tile([P, CO], f32)
            nc.sync.dma_start(bgcn_sb[:], bgcn_in[:])
            b1_sb = CPool.tile([P, CO], f32)
            nc.sync.dma_start(b1_sb[:], b1_in[:])
            b2_sb = CPool.tile([P, 1], f32)
            nc.sync.dma_start(b2_sb[:], b2_in[:])
            z_sb = CPool.tile([P, NB], f32)

            # dinv = rsqrt(deg): global (xs build) and own-blocks (drain)
            dega_sb = CPool.tile([P, NBLK], f32)
            nc.sync.dma_start(dega_sb[:], deg_in[:])
            dinv_sb = CPool.tile([P, NBLK], f32)
            nc.scalar.activation(dinv_sb[:], dega_sb[:], AF.Sqrt)
            nc.vector.reciprocal(dinv_sb[:], dinv_sb[:])
            dinva_f8 = CPool.tile([P, NBLK], fp8)
            nc.vector.tensor_copy(dinva_f8[:], dinv_sb[:])
            dego_sb = CPool.tile([P, NB], f32)
            nc.sync.dma_start(dego_sb[:], dego_in[:])
            dinvo_sb = CPool.tile([P, NB], f32)
            nc.scalar.activation(dinvo_sb[:], dego_sb[:], AF.Sqrt)
            nc.vector.reciprocal(dinvo_sb[:], dinvo_sb[:])

            # ------- xs = dinv * x (fp8), per sub-table; ACT-engine DMAs ----
            # (keeps the sync-engine HWDGE FIFO free for idx loads so the
            # first gathers are not queued behind the whole xs phase)
            with tc.tile_pool(name="xsp", bufs=4) as xsp:
                it = 0
                for qq in range(SUB):
                    for b0 in range(0, SUBBLK, XB):
                        gb0 = qq * SUBBLK + b0
                        xt = xsp.tile([P, XB, C], fp8, tag="xt")
                        nc.scalar.dma_start(
                            xt[:],
                            x_in[gb0 * P:(gb0 + XB) * P, :]
                            .rearrange("(j p) c -> p j c", p=P))
                        xf = xsp.tile([P, XB, C], fp8, tag="xf")
                        eng = nc.vector if it % 2 == 0 else nc.gpsimd
                        eng.tensor_tensor(
                            xf[:], xt[:],
                            dinva_f8[:, gb0:gb0 + XB, None]
                            .to_broadcast([P, XB, C]),
                            OP.mult)
                        nc.scalar.dma_start(
                            xs_q[qq][b0 * P:(b0 + XB) * P, :]
                            .rearrange("(j p) c -> p j c", p=P),
                            xf[:])
                        it += 1

            with tc.tile_pool(name="idxp", bufs=6) as idxp, \
                 tc.tile_pool(name="gbp", bufs=6) as gbp, \
                 tc.tile_pool(name="m16p", bufs=4) as m16p, \
                 tc.tile_pool(name="ybfp", bufs=2) as ybfp, \
                 tc.tile_pool(name="yTp", bufs=2) as yTp, \
                 tc.tile_pool(name="gTp", bufs=2) as gTp, \
                 tc.tile_pool(name="hTp", bufs=2) as hTp, \
                 tc.tile_pool(name="zrp", bufs=2) as zrp, \
                 tc.tile_pool(name="yps", bufs=GRP, space="PSUM") as ypsp, \
                 tc.tile_pool(name="mmp", bufs=1, space="PSUM") as mmp:

                qc = 0
                pending_mlp = None

                def emit_mlp(gg, yT):
                    gT = gTp.tile([P, CO, GRP, P], bf16, tag="gT")
                    hT = hTp.tile([P, CO, GRP, P], bf16, tag="hT")
                    for src, dst, wsb, bsb in ((yT, gT, wgcnT_sb, bgcn_sb),
                                               (gT, hT, w1T_sb, b1_sb)):
                        for oi in range(CO):
                            for j0, j1 in ((0, 4), (4, GRP)):
                                mm = mmp.tile([P, 4, P], f32, tag="mm")
                                cb = j1 - j0
                                for ci in range(CO):
                                    if src is yT:
                                        rhs = src[:, j0:j1, ci, :]
                                    else:
                                        rhs = src[:, ci, j0:j1, :]
                                    nc.tensor.matmul(
                                        mm[:, :cb, :],
                                        lhsT=wsb[:, ci, oi * P:(oi + 1) * P],
                                        rhs=rhs,
                                        start=(ci == 0), stop=(ci == CO - 1))
                                nc.scalar.activation(
                                    dst[:, oi, j0:j1, :], mm[:, :cb, :],
                                    AF.Relu, bias=bsb[:, oi:oi + 1])
                    zp = mmp.tile([P, GRP], f32, tag="mm")
                    for j in range(GRP):
                        for ci in range(CO):
                            nc.tensor.matmul(
                                zp[:, j:j + 1], lhsT=hT[:, ci, j, :],
                                rhs=w2col_sb[:, ci, :],
                                start=(ci == 0), stop=(ci == CO - 1))
                    zr = zrp.tile([P, GRP], f32, tag="zr")
                    nc.vector.tensor_scalar(zr[:], zp[:], b2_sb[:], 0.0,
                                            OP.add, OP.max)
                    nc.scalar.activation(z_sb[:, gg * GRP:(gg + 1) * GRP],
                                         zr[:], AF.Sigmoid)

                for gg in range(NGRP):
                    gbs = []
                    for qq in range(SUB):
                        gqi = gg * SUB + qq
                        tgq = int(T[gqi])
                        t0g = int(toff[gqi])
                        ib = idxp.tile([P, TGQMAX * 8], i16, tag="ib")
                        nc.sync.dma_start(ib[:, :tgq * 8],
                                          idx_in[:, t0g * 8:(t0g + tgq) * 8])
                        gb = gbp.tile([P, TGQMAX, C], fp8, tag="gb")
                        for c0 in range(0, tgq, GMAX):
                            cn = min(GMAX, tgq - c0)
                            nc.gpsimd.dma_gather(
                                gb[:, c0:c0 + cn, :],
                                xs_q[qq][:],
                                ib[:, c0 * 8:(c0 + cn) * 8],
                                num_idxs=cn * P,
                                num_idxs_reg=cn * P,
                                elem_size=C,
                                elem_step=C,
                                queue_num=qc % NQ,
                            )
                            qc += 1
                        gbs.append(gb)

                    ypt = [ypsp.tile([P, C], f32, tag="y", name=f"yp{j}")
                           for j in range(GRP)]
                    mbase, mcount = grp_nmm[gg]
                    for m0 in range(mbase, mbase + mcount, BCO):
                        m16 = m16p.tile([P, BCO, P], fp8, tag="c16")
                        nc.vector.tensor_tensor(
                            m16[:],
                            colrel_sb[:, m0:m0 + BCO, None].to_broadcast(
                                [P, BCO, P]),
                            iota_sb[:], OP.is_equal)
                        for mi in range(m0, m0 + BCO):
                            if e_gq[mi] < 0:
                                continue        # pad entry
                            qq = int(e_gq[mi]) - gg * SUB
                            nc.tensor.matmul(
                                ypt[int(e_s[mi])][:],
                                lhsT=m16[:, mi - m0, :],
                                rhs=gbs[qq][:, int(e_t[mi]), :],
                                start=bool(e_start[mi]),
                                stop=bool(e_stop[mi]))

                    ybf = ybfp.tile([P, GRP, C], bf16, tag="ybf")
                    for j in range(GRP):
                        nc.scalar.activation(
                            ybf[:, j, :], ypt[j][:], AF.Copy,
                            scale=dinvo_sb[:, gg * GRP + j:gg * GRP + j + 1])
                    yT = yTp.tile([P, GRP, CO, P], bf16, tag="yT")
                    for j in range(GRP):
                        for ci in range(CO):
                            # scalar (ACT) DGE: same-engine order after the
                            # drains; keeps the sync FIFO free for idx loads
                            nc.scalar.dma_start_transpose(
                                yT[:, j, ci, :],
                                ybf[:, j, ci * P:(ci + 1) * P])

                    if pending_mlp is not None:
                        emit_mlp(*pending_mlp)
                    pending_mlp = (gg, yT)

                emit_mlp(*pending_mlp)
                nc.sync.dma_start(z_out[:], z_sb[:])

    nc.compile()
    return nc


# ----------------------------------------------------------------------------
# entry point
# ----------------------------------------------------------------------------

def _install_ntff_hook():
    """Best-effort: register the axon NTFF profile hook so trace=True works."""
    import sys, types, contextlib, ctypes
    if "antenv.axon_hooks" in sys.modules:
        return True
    try:
        lib = ctypes.CDLL("/opt/axon/libaxon_pjrt.so")
        if not hasattr(lib, "axon_start_nrt_profile"):
            return False
        lib.axon_start_nrt_profile.argtypes = [ctypes.POINTER(ctypes.c_int64), ctypes.c_size_t]
        lib.axon_start_nrt_profile.restype = ctypes.c_int64
        lib.axon_stop_nrt_profile.argtypes = [ctypes.c_char_p]
        lib.axon_stop_nrt_profile.restype = ctypes.c_int64

        @contextlib.contextmanager
        def _hook(output_dir, device_ids):
            import jax
            jax.devices()
            if device_ids:
                ids = (ctypes.c_int64 * len(device_ids))(*device_ids)
                rc = lib.axon_start_nrt_profile(ids, len(device_ids))
            else:
                rc = lib.axon_start_nrt_profile(None, 0)
            if rc != 0:
                raise RuntimeError(f"axon_start_nrt_profile rc={rc}")
            try:
                yield
            finally:
                n = lib.axon_stop_nrt_profile(str(output_dir).encode())
                if n < 0:
                    raise RuntimeError(f"axon_stop_nrt_profile rc={n}")

        mod = types.ModuleType("antenv.axon_hooks")
        mod.get_axon_ntff_profile_hook = lambda: _hook
        mod.set_axon_ntff_profile_hook = lambda h: None
        sys.modules["antenv.axon_hooks"] = mod
        return True
    except Exception:
        return False


def kernel(x, edge_index, W_gcn, b_gcn, W1, b1, W2, b2, _trace=None):
    global LAST_EXEC_NS
    from concourse.bass_utils import run_bass_kernel_spmd

    x = np.asarray(x, dtype=np.float32)
    edge_index = np.asarray(edge_index)
    sched, idx_in, colrel_in, deg_tbl, deg_own = _preprocess(edge_index)
    wd = _prep_weights(W_gcn, b_gcn, W1, b1, W2, b2)

    x_pad = np.zeros((NPAD, C), dtype=_FP8)
    x_pad[:N] = x.astype(_FP8)

    nc = _build(sched)
    in_maps = []
    for k in range(NCORE):
        in_maps.append(dict(
            xb=x_pad,
            idx=np.ascontiguousarray(idx_in[k]),
            colrel=np.ascontiguousarray(colrel_in[k]),
            dega=deg_tbl,
            dego=np.ascontiguousarray(deg_own[k]),
            wgcnT=wd["wgcnT"], w1T=wd["w1T"], w2col=wd["w2col"],
            bgcn=wd["bgcn"], b1=wd["b1"],
            b2t=np.full((P, 1), wd["b2"], dtype=np.float32),
            iota16=wd["iota16"],
        ))

    trace = _trace if _trace is not None else _install_ntff_hook()
    res = run_bass_kernel_spmd(nc, in_maps, core_ids=list(range(NCORE)),
                               trace=bool(trace))
    LAST_EXEC_NS = res.exec_time_ns

    zs = []
    for k in range(NCORE):
        zk = np.asarray(res.results[k]["z"])          # [128, NB]
        zs.append(zk.T.reshape(-1))                   # node-major within core
    out = np.concatenate(zs)[:N].astype(np.float32).reshape(N, 1)
    return out


# revision 18
# speedup vs baseline: 1.6608x; 1.6608x over previous
"""GCN message-passing kernel for 8 TRN2 NeuronCores (Bass/Tile), v4.

Math (equivalent to the PyG-style reference):
    deg[i]  = 1 + #{edges with target i}              (self-loops added)
    dinv    = deg^-1/2
    y[i]    = dinv[i] * sum_{j -> i} dinv[j] * x[j]   (incl. self loop j=i)
    g       = relu(y @ Wg^T + bg)
    h       = relu(g @ W1^T + b1)
    out     = sigmoid(relu(h @ W2^T + b2))

Design notes:
  - Host does index work only: shard/sort/pad edges, count degrees.
    Device computes dinv = rsqrt(deg), the scaled table xs = dinv*x (fp8),
    all matmuls and activations.
  - Nodes sharded: core k owns 98 blocks of 128 targets, grouped into 17
    grps of <=6 (one PSUM bank per block accumulator; 2 banks left for the
    MLP and TensorE transposes).
  - Edges sorted by (grp, src sub-table, target block); each (grp, sub,
    block) segment is padded to a cross-core common length so one SPMD
    program serves all cores (colrel sentinels mask absent rows).
  - dma_gather pulls xs rows in 1024-idx calls cycled over 4 SWDGE queues
    (the Q7 descriptor-generation rate, ~2.8ns/idx with 4 queues, is the
    kernel's critical resource; single-queue is 3x slower).
  - Steady-state HWDGE traffic is ONE idx load per grp: everything else
    (transposes, drains, MLP) stays on-chip, because HWDGE completion
    semaphore lanes are shared across engines and queued DMAs would
    false-serialize the gather stream.
  - Aggregation: per 128-edge tile, a pure one-hot fp8 matrix (one DVE
    is_equal from a colrel table) scatters gathered rows into the block's
    PSUM via TensorE matmul; dinv[tgt] rides the ACT drain (Copy*scale).
"""

import math

import numpy as np
import ml_dtypes

P = 128
C = 256
CO = 2                      # C // P
NCORE = 8
N = 100000
NB = 98                     # blocks per core
NBLK = NB * NCORE           # 784
NPAD = NBLK * P             # 100352
GRP = 6                     # max blocks per psum group
GRP_SIZES = [6] * 16 + [2]  # 17 grps covering 98 blocks
NGRP = len(GRP_SIZES)
SUB = 4                     # gather sub-tables (int16 index limit)
SUBROWS = NPAD // SUB       # 25088
SUBBLK = SUBROWS // P       # 196
NGQ = NGRP * SUB
GMAX = 8                    # tiles per dma_gather call (1024-idx ring limit)
NQ = 4                      # SWDGE queues
BCO = 16                    # coeff tiles built per DVE instruction
XB = 7                      # x blocks per xs-build step

_BF16 = ml_dtypes.bfloat16
_FP8 = ml_dtypes.float8_e4m3

LAST_EXEC_NS = None


# ----------------------------------------------------------------------------
# host-side preprocessing (index work: shard, sort, pad, count)
# ----------------------------------------------------------------------------

def _preprocess(edge_index):
    row = np.ascontiguousarray(edge_index[0]).astype(np.int64)
    col = np.ascontiguousarray(edge_index[1]).astype(np.int64)
    loop = np.arange(N, dtype=np.int64)
    row = np.concatenate([row, loop])
    col = np.concatenate([col, loop])

    deg = np.bincount(col, minlength=NPAD).astype(np.float32)
    deg[N:] = 1.0
    deg_tbl = np.ascontiguousarray(deg.reshape(NBLK, P).T)      # [P, NBLK]

    blk = col >> 7
    core = blk // NB
    bl = blk - core * NB
    g = bl // GRP
    s = bl % GRP
    q = row // SUBROWS
    gq = g * SUB + q

    order = np.lexsort((col, gq, core))
    row_s = row[order]
    col_s = col[order]
    core_s = core[order]
    q_s = q[order]

    # segment = (core, gq, slot); common padded length per (gq, slot)
    gqs = (gq * GRP + s)
    seg = core * (NGQ * GRP) + gqs
    seg_s = seg[order]
    cnt3 = np.bincount(seg, minlength=NCORE * NGQ * GRP).reshape(NCORE, -1)
    S = cnt3.max(axis=0).reshape(NGQ, GRP)                      # [NGQ, GRP]
    soff = np.zeros((NGQ, GRP + 1), np.int64)
    np.cumsum(S, axis=1, out=soff[:, 1:])
    L = soff[:, -1]                                             # edges per gq
    T = (L + P - 1) // P                                        # tiles per gq
    toff = np.zeros(NGQ + 1, np.int64)
    np.cumsum(T, out=toff[1:])
    NTILE = int(toff[-1])
    NIDX = NTILE * P

    # entry schedule: (gq, tile, slots overlapping) in order; BCO-pad per grp
    Tmax = int(T.max())
    lut = np.full((NGQ, Tmax, GRP), -1, np.int64)
    e_gq, e_t, e_s = [], [], []
    e_start, e_stop = [], []
    seen_start = set()
    grp_nmm = []
    m = 0
    for gg in range(NGRP):
        m0 = m
        nslot = GRP_SIZES[gg]
        for qq in range(SUB):
            gqi = gg * SUB + qq
            t_lo = soff[gqi, :-1] // P
            t_hi = (soff[gqi, 1:] - 1) // P + 1
            for t in range(int(T[gqi])):
                for ss in range(nslot):
                    if S[gqi, ss] > 0 and t_lo[ss] <= t < t_hi[ss]:
                        lut[gqi, t, ss] = m
                        e_gq.append(gqi); e_t.append(t); e_s.append(ss)
                        key = (gg, ss)
                        e_start.append(key not in seen_start)
                        seen_start.add(key)
                        e_stop.append(False)
                        m += 1
        seen = set()
        for i in range(m - 1, m0 - 1, -1):
            if e_s[i] not in seen:
                e_stop[i] = True
                seen.add(e_s[i])
                if len(seen) == nslot:
                    break
        npad = (-(m - m0)) % BCO
        for _ in range(npad):
            e_gq.append(-1); e_t.append(0); e_s.append(0)
            e_start.append(False); e_stop.append(False)
        m += npad
        grp_nmm.append((m0, m - m0))
    NMM = m
    e_gq = np.array(e_gq); e_t = np.array(e_t); e_s = np.array(e_s)
    e_start = np.array(e_start); e_stop = np.array(e_stop)

    # per-core data tables
    idx_dat = np.zeros((NCORE, NIDX), np.int16)
    colrel_dat = np.full((NCORE, NMM * P), 254.0, np.float32)

    seg_start = np.zeros(NCORE * NGQ * GRP + 1, np.int64)
    np.cumsum(cnt3.reshape(-1), out=seg_start[1:])
    pos_in_seg = np.arange(len(row_s), dtype=np.int64) - seg_start[seg_s]
    gqs_s = gqs[order]
    pos_in_gq = soff[:, :-1].reshape(-1)[gqs_s] + pos_in_seg    # within gq
    tile_e = pos_in_gq // P
    p_e = pos_in_gq % P
    gpos = toff[gqs_s // GRP] * P + pos_in_gq
    m_e = lut[gqs_s // GRP, tile_e, gqs_s % GRP]
    assert (m_e >= 0).all()
    idx_val = (row_s - q_s * SUBROWS).astype(np.int16)
    colv = (col_s & 127).astype(np.float32)
    flat_m = m_e * P + p_e
    for k in range(NCORE):
        sel = core_s == k
        idx_dat[k, gpos[sel]] = idx_val[sel]
        colrel_dat[k, flat_m[sel]] = colv[sel]

    idx_in = np.ascontiguousarray(
        np.tile(idx_dat.reshape(NCORE, NIDX // 16, 16).transpose(0, 2, 1),
                (1, 8, 1)))                                 # [NCORE,128,NIDX//16]
    colrel_in = np.ascontiguousarray(
        colrel_dat.reshape(NCORE, NMM, P).transpose(0, 2, 1)).astype(_BF16)

    deg_own = np.stack([deg_tbl[:, k * NB:(k + 1) * NB] for k in range(NCORE)])

    sched = dict(T=T, toff=toff, NTILE=NTILE, NMM=NMM, grp_nmm=grp_nmm,
                 e_gq=e_gq, e_t=e_t, e_s=e_s, e_start=e_start, e_stop=e_stop)
    return sched, idx_in, colrel_in, deg_tbl, np.ascontiguousarray(deg_own)


def _prep_weights(W_gcn, b_gcn, W1, b1, W2, b2):
    def wT(W):  # [C,C] -> lhsT layout [128, CO, C]: [p, ci, o] = W[o, ci*128+p]
        return np.ascontiguousarray(
            np.asarray(W).T.reshape(CO, P, C).transpose(1, 0, 2)).astype(_BF16)
    w2col = np.ascontiguousarray(
        np.asarray(W2).reshape(C).reshape(CO, P).transpose(1, 0)[:, :, None]
    ).astype(_BF16)
    bg = np.ascontiguousarray(np.asarray(b_gcn).reshape(CO, P).T).astype(np.float32)
    bb1 = np.ascontiguousarray(np.asarray(b1).reshape(CO, P).T).astype(np.float32)
    iota16 = np.tile(np.arange(P, dtype=np.float32), (P, BCO)).astype(_BF16)
    ident = np.eye(P, dtype=np.float32).astype(_BF16)
    return dict(wgcnT=wT(W_gcn), w1T=wT(W1), w2col=w2col, bgcn=bg, b1=bb1,
                b2=float(np.asarray(b2).reshape(-1)[0]), iota16=iota16,
                ident=ident)


# ----------------------------------------------------------------------------
# device program (SPMD: one program, 8 cores; per-core data differs)
# ----------------------------------------------------------------------------

def _build(sched):
    from concourse import bacc, mybir
    from concourse import tile as ctile

    T = sched["T"]
    toff = sched["toff"]
    NTILE = sched["NTILE"]
    NMM = sched["NMM"]
    grp_nmm = sched["grp_nmm"]
    e_gq = sched["e_gq"]
    e_t = sched["e_t"]
    e_s = sched["e_s"]
    e_start = sched["e_start"]
    e_stop = sched["e_stop"]
    TGQMAX = int(T.max())
    TGMAX = int(max(toff[(gg + 1) * SUB] - toff[gg * SUB]
                    for gg in range(NGRP)))

    f32 = mybir.dt.float32
    bf16 = mybir.dt.bfloat16
    fp8 = mybir.dt.float8e4
    i16 = mybir.dt.int16
    AF = mybir.ActivationFunctionType
    OP = mybir.AluOpType

    nc = bacc.Bacc(None, target_bir_lowering=False, debug=False,
                   num_devices=NCORE, num_swdge_queues=NQ)

    x_in = nc.dram_tensor("xb", [NPAD, C], fp8, kind="ExternalInput")
    idx_in = nc.dram_tensor("idx", [P, NTILE * 8], i16, kind="ExternalInput")
    colrel_in = nc.dram_tensor("colrel", [P, NMM], bf16, kind="ExternalInput")
    deg_in = nc.dram_tensor("dega", [P, NBLK], f32, kind="ExternalInput")
    dego_in = nc.dram_tensor("dego", [P, NB], f32, kind="ExternalInput")
    wgcnT_in = nc.dram_tensor("wgcnT", [P, CO, C], bf16, kind="ExternalInput")
    w1T_in = nc.dram_tensor("w1T", [P, CO, C], bf16, kind="ExternalInput")
    w2col_in = nc.dram_tensor("w2col", [P, CO, 1], bf16, kind="ExternalInput")
    bgcn_in = nc.dram_tensor("bgcn", [P, CO], f32, kind="ExternalInput")
    b1_in = nc.dram_tensor("b1", [P, CO], f32, kind="ExternalInput")
    b2_in = nc.dram_tensor("b2t", [P, 1], f32, kind="ExternalInput")
    iota_in = nc.dram_tensor("iota16", [P, BCO * P], bf16, kind="ExternalInput")
    ident_in = nc.dram_tensor("ident", [P, P], bf16, kind="ExternalInput")

    z_out = nc.dram_tensor("z", [P, NB], f32, kind="ExternalOutput")
    xs_q = [nc.dram_tensor(f"xs{q}", [SUBROWS, C], fp8) for q in range(SUB)]

    with ctile.TileContext(nc) as tc:
        with tc.tile_pool(name="const", bufs=1) as CPool:
            colrel_sb = CPool.tile([P, NMM], bf16)
            nc.sync.dma_start(colrel_sb[:], colrel_in[:])
            iota_sb = CPool.tile([P, BCO, P], bf16)
            nc.sync.dma_start(iota_sb[:],
                              iota_in[:].rearrange("p (j f) -> p j f", f=P))
            ident_sb = CPool.tile([P, P], bf16)
            nc.sync.dma_start(ident_sb[:], ident_in[:])
            wgcnT_sb = CPool.tile([P, CO, C], bf16)
            nc.sync.dma_start(wgcnT_sb[:], wgcnT_in[:])
            w1T_sb = CPool.tile([P, CO, C], bf16)
            nc.sync.dma_start(w1T_sb[:], w1T_in[:])
            w2col_sb = CPool.tile([P, CO, 1], bf16)
            nc.sync.dma_start(w2col_sb[:], w2col_in[:])
            bgcn_sb = CPool.tile([P, CO], f32)
            nc.sync.dma_start(bgcn_sb[:], bgcn_in[:])
            b1_sb = CPool.tile([P, CO], f32)
            nc.sync.dma_start(b1_sb[:], b1_in[:])
            b2_sb = CPool.tile([P, 1], f32)
            nc.sync.dma_start(b2_sb[:], b2_in[:])
            z_sb = CPool.tile([P, NB], f32)

            # dinv = rsqrt(deg): global (xs build) and own-blocks (drain)
            dega_sb = CPool.tile([P, NBLK], f32)
            nc.sync.dma_start(dega_sb[:], deg_in[:])
            dinv_sb = CPool.tile([P, NBLK], f32)
            nc.scalar.activation(dinv_sb[:], dega_sb[:], AF.Sqrt)
            nc.vector.reciprocal(dinv_sb[:], dinv_sb[:])
            dinva_f8 = CPool.tile([P, NBLK], fp8)
            nc.vector.tensor_copy(dinva_f8[:], dinv_sb[:])
            dego_sb = CPool.tile([P, NB], f32)
            nc.sync.dma_start(dego_sb[:], dego_in[:])
            dinvo_sb = CPool.tile([P, NB], f32)
            nc.scalar.activation(dinvo_sb[:], dego_sb[:], AF.Sqrt)
            nc.vector.reciprocal(dinvo_sb[:], dinvo_sb[:])

            # ------- xs = dinv * x (fp8), per sub-table; ACT-engine DMAs ----
            # mults alternate DVE/GpSimd to halve the phase latency
            with tc.tile_pool(name="xsp", bufs=4) as xsp:
                it = 0
                for qq in range(SUB):
                    for b0 in range(0, SUBBLK, XB):
                        gb0 = qq * SUBBLK + b0
                        xt = xsp.tile([P, XB, C], fp8, tag="xt")
                        nc.scalar.dma_start(
                            xt[:],
                            x_in[gb0 * P:(gb0 + XB) * P, :]
                            .rearrange("(j p) c -> p j c", p=P))
                        xf = xsp.tile([P, XB, C], fp8, tag="xf")
                        eng = nc.vector if it % 2 == 0 else nc.gpsimd
                        eng.tensor_tensor(
                            xf[:], xt[:],
                            dinva_f8[:, gb0:gb0 + XB, None]
                            .to_broadcast([P, XB, C]),
                            OP.mult)
                        nc.scalar.dma_start(
                            xs_q[qq][b0 * P:(b0 + XB) * P, :]
                            .rearrange("(j p) c -> p j c", p=P),
                            xf[:])
                        it += 1

            with tc.tile_pool(name="idxp", bufs=3) as idxp, \
                 tc.tile_pool(name="gbp", bufs=6) as gbp, \
                 tc.tile_pool(name="m16p", bufs=4) as m16p, \
                 tc.tile_pool(name="ybfp", bufs=2) as ybfp, \
                 tc.tile_pool(name="yTp", bufs=2) as yTp, \
                 tc.tile_pool(name="gTp", bufs=2) as gTp, \
                 tc.tile_pool(name="hTp", bufs=2) as hTp, \
                 tc.tile_pool(name="zrp", bufs=2) as zrp, \
                 tc.tile_pool(name="yps", bufs=GRP, space="PSUM") as ypsp, \
                 tc.tile_pool(name="tpp", bufs=1, space="PSUM") as tpp, \
                 tc.tile_pool(name="mmp", bufs=1, space="PSUM") as mmp:

                qc = 0
                pending_mlp = None

                def emit_mlp(gg, gsz, yT):
                    b0blk = sum(GRP_SIZES[:gg])
                    chunks = [(0, min(4, gsz))] + ([(4, gsz)] if gsz > 4 else [])
                    gT = gTp.tile([P, CO, GRP, P], bf16, tag="gT")
                    hT = hTp.tile([P, CO, GRP, P], bf16, tag="hT")
                    for src, dst, wsb, bsb in ((yT, gT, wgcnT_sb, bgcn_sb),
                                               (gT, hT, w1T_sb, b1_sb)):
                        for oi in range(CO):
                            for j0, j1 in chunks:
                                mm = mmp.tile([P, 4, P], f32, tag="mm")
                                cb = j1 - j0
                                for ci in range(CO):
                                    if src is yT:
                                        rhs = src[:, j0:j1, ci, :]
                                    else:
                                        rhs = src[:, ci, j0:j1, :]
                                    nc.tensor.matmul(
                                        mm[:, :cb, :],
                                        lhsT=wsb[:, ci, oi * P:(oi + 1) * P],
                                        rhs=rhs,
                                        start=(ci == 0), stop=(ci == CO - 1))
                                nc.scalar.activation(
                                    dst[:, oi, j0:j1, :], mm[:, :cb, :],
                                    AF.Relu, bias=bsb[:, oi:oi + 1])
                    zp = mmp.tile([P, GRP], f32, tag="mm")
                    for j in range(gsz):
                        for ci in range(CO):
                            nc.tensor.matmul(
                                zp[:, j:j + 1], lhsT=hT[:, ci, j, :],
                                rhs=w2col_sb[:, ci, :],
                                start=(ci == 0), stop=(ci == CO - 1))
                    zr = zrp.tile([P, GRP], f32, tag="zr")
                    nc.vector.tensor_scalar(zr[:, :gsz], zp[:, :gsz],
                                            b2_sb[:], 0.0, OP.add, OP.max)
                    nc.scalar.activation(z_sb[:, b0blk:b0blk + gsz],
                                         zr[:, :gsz], AF.Sigmoid)

                for gg in range(NGRP):
                    gsz = GRP_SIZES[gg]
                    b0blk = sum(GRP_SIZES[:gg])
                    tg0 = int(toff[gg * SUB])
                    tgn = int(toff[(gg + 1) * SUB]) - tg0
                    ib = idxp.tile([P, TGMAX * 8], i16, tag="ib")
                    nc.sync.dma_start(ib[:, :tgn * 8],
                                      idx_in[:, tg0 * 8:(tg0 + tgn) * 8])
                    gbs = []
                    for qq in range(SUB):
                        gqi = gg * SUB + qq
                        tgq = int(T[gqi])
                        tb0 = int(toff[gqi]) - tg0      # tile base within ib
                        gb = gbp.tile([P, TGQMAX, C], fp8, tag="gb")
                        for c0 in range(0, tgq, GMAX):
                            cn = min(GMAX, tgq - c0)
                            nc.gpsimd.dma_gather(
                                gb[:, c0:c0 + cn, :],
                                xs_q[qq][:],
                                ib[:, (tb0 + c0) * 8:(tb0 + c0 + cn) * 8],
                                num_idxs=cn * P,
                                num_idxs_reg=cn * P,
                                elem_size=C,
                                elem_step=C,
                                queue_num=qc % NQ,
                            )
                            qc += 1
                        gbs.append(gb)

                    ypt = [ypsp.tile([P, C], f32, tag="y", name=f"yp{j}")
                           for j in range(gsz)]
                    mbase, mcount = grp_nmm[gg]
                    for m0 in range(mbase, mbase + mcount, BCO):
                        m16 = m16p.tile([P, BCO, P], fp8, tag="c16")
                        nc.vector.tensor_tensor(
                            m16[:],
                            colrel_sb[:, m0:m0 + BCO, None].to_broadcast(
                                [P, BCO, P]),
                            iota_sb[:], OP.is_equal)
                        for mi in range(m0, m0 + BCO):
                            if e_gq[mi] < 0:
                                continue        # pad entry
                            qq = int(e_gq[mi]) - gg * SUB
                            nc.tensor.matmul(
                                ypt[int(e_s[mi])][:],
                                lhsT=m16[:, mi - m0, :],
                                rhs=gbs[qq][:, int(e_t[mi]), :],
                                start=bool(e_start[mi]),
                                stop=bool(e_stop[mi]))

                    # drain y (ACT: cast + dinv[tgt] scale)
                    ybf = ybfp.tile([P, GRP, C], bf16, tag="ybf")
                    for j in range(gsz):
                        nc.scalar.activation(
                            ybf[:, j, :], ypt[j][:], AF.Copy,
                            scale=dinvo_sb[:, b0blk + j:b0blk + j + 1])

                    # MLP of the previous grp overlaps this grp's transposes
                    if pending_mlp is not None:
                        emit_mlp(*pending_mlp)

                    # transpose y via TensorE (no DMA: keeps HWDGE lanes free)
                    yT = yTp.tile([P, GRP, CO, P], bf16, tag="yT")
                    for j in range(gsz):
                        for ci in range(CO):
                            tp = tpp.tile([P, P], bf16, tag="tp")
                            nc.tensor.transpose(
                                tp[:], ybf[:, j, ci * P:(ci + 1) * P],
                                ident_sb[:])
                            nc.scalar.activation(yT[:, j, ci, :], tp[:],
                                                 AF.Copy)
                    pending_mlp = (gg, gsz, yT)

                emit_mlp(*pending_mlp)
                nc.sync.dma_start(z_out[:], z_sb[:])

    nc.compile()
    return nc


# ----------------------------------------------------------------------------
# entry point
# ----------------------------------------------------------------------------

def _install_ntff_hook():
    """Best-effort: register the axon NTFF profile hook so trace=True works."""
    import sys, types, contextlib, ctypes
    if "antenv.axon_hooks" in sys.modules:
        return True
    try:
        lib = ctypes.CDLL("/opt/axon/libaxon_pjrt.so")
        if not hasattr(lib, "axon_start_nrt_profile"):
            return False
        lib.axon_start_nrt_profile.argtypes = [ctypes.POINTER(ctypes.c_int64), ctypes.c_size_t]
        lib.axon_start_nrt_profile.restype = ctypes.c_int64
        lib.axon_stop_nrt_profile.argtypes = [ctypes.c_char_p]
        lib.axon_stop_nrt_profile.restype = ctypes.c_int64

        @contextlib.contextmanager
        def _hook(output_dir, device_ids):
            import jax
            jax.devices()
            if device_ids:
                ids = (ctypes.c_int64 * len(device_ids))(*device_ids)
                rc = lib.axon_start_nrt_profile(ids, len(device_ids))
            else:
                rc = lib.axon_start_nrt_profile(None, 0)
            if rc != 0:
                raise RuntimeError(f"axon_start_nrt_profile rc={rc}")
            try:
                yield
            finally:
                n = lib.axon_stop_nrt_profile(str(output_dir).encode())
                if n < 0:
                    raise RuntimeError(f"axon_stop_nrt_profile rc={n}")

        mod = types.ModuleType("antenv.axon_hooks")
        mod.get_axon_ntff_profile_hook = lambda: _hook
        mod.set_axon_ntff_profile_hook = lambda h: None
        sys.modules["antenv.axon_hooks"] = mod
        return True
    except Exception:
        return False


def kernel(x, edge_index, W_gcn, b_gcn, W1, b1, W2, b2, _trace=None):
    global LAST_EXEC_NS
    from concourse.bass_utils import run_bass_kernel_spmd

    x = np.asarray(x, dtype=np.float32)
    edge_index = np.asarray(edge_index)
    sched, idx_in, colrel_in, deg_tbl, deg_own = _preprocess(edge_index)
    wd = _prep_weights(W_gcn, b_gcn, W1, b1, W2, b2)

    x_pad = np.zeros((NPAD, C), dtype=_FP8)
    x_pad[:N] = x.astype(_FP8)

    nc = _build(sched)
    in_maps = []
    for k in range(NCORE):
        in_maps.append(dict(
            xb=x_pad,
            idx=np.ascontiguousarray(idx_in[k]),
            colrel=np.ascontiguousarray(colrel_in[k]),
            dega=deg_tbl,
            dego=np.ascontiguousarray(deg_own[k]),
            wgcnT=wd["wgcnT"], w1T=wd["w1T"], w2col=wd["w2col"],
            bgcn=wd["bgcn"], b1=wd["b1"],
            b2t=np.full((P, 1), wd["b2"], dtype=np.float32),
            iota16=wd["iota16"], ident=wd["ident"],
        ))

    trace = _trace if _trace is not None else _install_ntff_hook()
    res = run_bass_kernel_spmd(nc, in_maps, core_ids=list(range(NCORE)),
                               trace=bool(trace))
    LAST_EXEC_NS = res.exec_time_ns

    zs = []
    for k in range(NCORE):
        zk = np.asarray(res.results[k]["z"])          # [128, NB]
        zs.append(zk.T.reshape(-1))                   # node-major within core
    out = np.concatenate(zs)[:N].astype(np.float32).reshape(N, 1)
    return out


# revision 27
# speedup vs baseline: 1.7685x; 1.0649x over previous
"""GCN message-passing kernel for 8 TRN2 NeuronCores (Bass/Tile), v4.

Math (equivalent to the PyG-style reference):
    deg[i]  = 1 + #{edges with target i}              (self-loops added)
    dinv    = deg^-1/2
    y[i]    = dinv[i] * sum_{j -> i} dinv[j] * x[j]   (incl. self loop j=i)
    g       = relu(y @ Wg^T + bg)
    h       = relu(g @ W1^T + b1)
    out     = sigmoid(relu(h @ W2^T + b2))

Design notes:
  - Host does index work only: shard/sort/pad edges, count degrees.
    Device computes dinv = rsqrt(deg), the scaled table xs = dinv*x (fp8),
    all matmuls and activations.
  - Nodes sharded: core k owns 98 blocks of 128 targets, grouped into 17
    grps of <=6 (one PSUM bank per block accumulator; 2 banks left for the
    MLP and TensorE transposes).
  - Edges sorted by (grp, src sub-table, target block); each (grp, sub,
    block) segment is padded to a cross-core common length so one SPMD
    program serves all cores (colrel sentinels mask absent rows).
  - dma_gather pulls xs rows in 1024-idx calls cycled over 4 SWDGE queues
    (the Q7 descriptor-generation rate, ~2.8ns/idx with 4 queues, is the
    kernel's critical resource; single-queue is 3x slower).
  - Steady-state HWDGE traffic is ONE idx load per grp: everything else
    (transposes, drains, MLP) stays on-chip, because HWDGE completion
    semaphore lanes are shared across engines and queued DMAs would
    false-serialize the gather stream.
  - Aggregation: per 128-edge tile, a pure one-hot fp8 matrix (one DVE
    is_equal from a colrel table) scatters gathered rows into the block's
    PSUM via TensorE matmul; dinv[tgt] rides the ACT drain (Copy*scale).
"""

import math

import numpy as np
import ml_dtypes

P = 128
C = 256
CO = 2                      # C // P
NCORE = 8
N = 100000
NB = 98                     # blocks per core
NBLK = NB * NCORE           # 784
NPAD = NBLK * P             # 100352
GRP = 6                     # max blocks per psum group
GRP_SIZES = [6] * 16 + [2]  # 17 grps covering 98 blocks
NGRP = len(GRP_SIZES)
SUB = 4                     # gather sub-tables (int16 index limit)
SUBROWS = NPAD // SUB       # 25088
SUBBLK = SUBROWS // P       # 196
NGQ = NGRP * SUB
GMAX = 8                    # tiles per dma_gather call (1024-idx ring limit)
NQ = 4                      # SWDGE queues
BCO = 16                    # coeff tiles built per DVE instruction
XB = 7                      # x blocks per xs-build step

_BF16 = ml_dtypes.bfloat16
_FP8 = ml_dtypes.float8_e4m3

LAST_EXEC_NS = None


# ----------------------------------------------------------------------------
# host-side preprocessing (index work: shard, sort, pad, count)
# ----------------------------------------------------------------------------

def _preprocess(edge_index):
    row = np.ascontiguousarray(edge_index[0]).astype(np.int64)
    col = np.ascontiguousarray(edge_index[1]).astype(np.int64)
    loop = np.arange(N, dtype=np.int64)
    row = np.concatenate([row, loop])
    col = np.concatenate([col, loop])

    deg = np.bincount(col, minlength=NPAD).astype(np.float32)
    deg[N:] = 1.0
    deg_tbl = np.ascontiguousarray(deg.reshape(NBLK, P).T)      # [P, NBLK]

    # assign target blocks to (core, local slot) by degree rank so the 8
    # blocks sharing a (grp, slot, sub) segment have near-equal edge counts
    # (the segment is padded to the max over cores - matched ranks minimize
    # that padding and balance total load)
    blkdeg = deg.reshape(NBLK, P).sum(axis=1)
    rank_order = np.argsort(-blkdeg, kind="stable")             # rank -> glb
    perm_core = np.empty(NBLK, np.int64)
    perm_loc = np.empty(NBLK, np.int64)
    perm_core[rank_order] = np.arange(NBLK) % NCORE
    perm_loc[rank_order] = np.arange(NBLK) // NCORE

    blk = col >> 7
    core = perm_core[blk]
    bl = perm_loc[blk]
    g = bl // GRP
    s = bl % GRP
    q = row // SUBROWS
    gq = g * SUB + q

    order = np.lexsort((col, s, gq, core))
    row_s = row[order]
    col_s = col[order]
    core_s = core[order]
    q_s = q[order]

    # segment = (core, gq, slot); common padded length per (gq, slot)
    gqs = (gq * GRP + s)
    seg = core * (NGQ * GRP) + gqs
    seg_s = seg[order]
    cnt3 = np.bincount(seg, minlength=NCORE * NGQ * GRP).reshape(NCORE, -1)
    S = cnt3.max(axis=0).reshape(NGQ, GRP)                      # [NGQ, GRP]
    soff = np.zeros((NGQ, GRP + 1), np.int64)
    np.cumsum(S, axis=1, out=soff[:, 1:])
    L = soff[:, -1]                                             # edges per gq
    T = (L + P - 1) // P                                        # tiles per gq
    toff = np.zeros(NGQ + 1, np.int64)
    np.cumsum(T, out=toff[1:])
    NTILE = int(toff[-1])
    NIDX = NTILE * P

    # entry schedule: (gq, tile, slots overlapping) in order; BCO-pad per grp
    Tmax = int(T.max())
    lut = np.full((NGQ, Tmax, GRP), -1, np.int64)
    e_gq, e_t, e_s = [], [], []
    e_start, e_stop = [], []
    seen_start = set()
    grp_nmm = []
    m = 0
    for gg in range(NGRP):
        m0 = m
        nslot = GRP_SIZES[gg]
        for qq in range(SUB):
            gqi = gg * SUB + qq
            t_lo = soff[gqi, :-1] // P
            t_hi = (soff[gqi, 1:] - 1) // P + 1
            for t in range(int(T[gqi])):
                for ss in range(nslot):
                    if S[gqi, ss] > 0 and t_lo[ss] <= t < t_hi[ss]:
                        lut[gqi, t, ss] = m
                        e_gq.append(gqi); e_t.append(t); e_s.append(ss)
                        key = (gg, ss)
                        e_start.append(key not in seen_start)
                        seen_start.add(key)
                        e_stop.append(False)
                        m += 1
        seen = set()
        for i in range(m - 1, m0 - 1, -1):
            if e_s[i] not in seen:
                e_stop[i] = True
                seen.add(e_s[i])
                if len(seen) == nslot:
                    break
        npad = (-(m - m0)) % BCO
        for _ in range(npad):
            e_gq.append(-1); e_t.append(0); e_s.append(0)
            e_start.append(False); e_stop.append(False)
        m += npad
        grp_nmm.append((m0, m - m0))
    NMM = m
    e_gq = np.array(e_gq); e_t = np.array(e_t); e_s = np.array(e_s)
    e_start = np.array(e_start); e_stop = np.array(e_stop)

    # per-core data tables
    idx_dat = np.zeros((NCORE, NIDX), np.int16)
    colrel_dat = np.full((NCORE, NMM * P), 254.0, np.float32)

    seg_start = np.zeros(NCORE * NGQ * GRP + 1, np.int64)
    np.cumsum(cnt3.reshape(-1), out=seg_start[1:])
    pos_in_seg = np.arange(len(row_s), dtype=np.int64) - seg_start[seg_s]
    gqs_s = gqs[order]
    pos_in_gq = soff[:, :-1].reshape(-1)[gqs_s] + pos_in_seg    # within gq
    tile_e = pos_in_gq // P
    p_e = pos_in_gq % P
    gpos = toff[gqs_s // GRP] * P + pos_in_gq
    m_e = lut[gqs_s // GRP, tile_e, gqs_s % GRP]
    assert (m_e >= 0).all()
    idx_val = (row_s - q_s * SUBROWS).astype(np.int16)
    colv = (col_s & 127).astype(np.float32)   # within-block target, unpermuted
    flat_m = m_e * P + p_e
    for k in range(NCORE):
        sel = core_s == k
        idx_dat[k, gpos[sel]] = idx_val[sel]
        colrel_dat[k, flat_m[sel]] = colv[sel]

    idx_in = np.ascontiguousarray(
        np.tile(idx_dat.reshape(NCORE, NIDX // 16, 16).transpose(0, 2, 1),
                (1, 8, 1)))                                 # [NCORE,128,NIDX//16]
    colrel_in = np.ascontiguousarray(
        colrel_dat.reshape(NCORE, NMM, P).transpose(0, 2, 1)).astype(_BF16)

    own_glb = np.stack([rank_order[np.arange(NB) * NCORE + k]
                        for k in range(NCORE)])                  # [NCORE, NB]
    deg_own = np.stack([deg_tbl[:, own_glb[k]] for k in range(NCORE)])

    sched = dict(T=T, toff=toff, NTILE=NTILE, NMM=NMM, grp_nmm=grp_nmm,
                 e_gq=e_gq, e_t=e_t, e_s=e_s, e_start=e_start, e_stop=e_stop,
                 own_glb=own_glb)
    return sched, idx_in, colrel_in, deg_tbl, np.ascontiguousarray(deg_own)


def _prep_weights(W_gcn, b_gcn, W1, b1, W2, b2):
    def wT(W):  # [C,C] -> lhsT layout [128, CO, C]: [p, ci, o] = W[o, ci*128+p]
        return np.ascontiguousarray(
            np.asarray(W).T.reshape(CO, P, C).transpose(1, 0, 2)).astype(_BF16)
    w2col = np.ascontiguousarray(
        np.asarray(W2).reshape(C).reshape(CO, P).transpose(1, 0)[:, :, None]
    ).astype(_BF16)
    bg = np.ascontiguousarray(np.asarray(b_gcn).reshape(CO, P).T).astype(np.float32)
    bb1 = np.ascontiguousarray(np.asarray(b1).reshape(CO, P).T).astype(np.float32)
    iota16 = np.tile(np.arange(P, dtype=np.float32), (P, BCO)).astype(_BF16)
    ident = np.eye(P, dtype=np.float32).astype(_BF16)
    return dict(wgcnT=wT(W_gcn), w1T=wT(W1), w2col=w2col, bgcn=bg, b1=bb1,
                b2=float(np.asarray(b2).reshape(-1)[0]), iota16=iota16,
                ident=ident)


# ----------------------------------------------------------------------------
# device program (SPMD: one program, 8 cores; per-core data differs)
# ----------------------------------------------------------------------------

def _build(sched):
    from concourse import bacc, mybir
    from concourse import tile as ctile

    T = sched["T"]
    toff = sched["toff"]
    NTILE = sched["NTILE"]
    NMM = sched["NMM"]
    grp_nmm = sched["grp_nmm"]
    e_gq = sched["e_gq"]
    e_t = sched["e_t"]
    e_s = sched["e_s"]
    e_start = sched["e_start"]
    e_stop = sched["e_stop"]
    TGQMAX = int(T.max())
    TGMAX = int(max(toff[(gg + 1) * SUB] - toff[gg * SUB]
                    for gg in range(NGRP)))

    f32 = mybir.dt.float32
    bf16 = mybir.dt.bfloat16
    fp8 = mybir.dt.float8e4
    i16 = mybir.dt.int16
    AF = mybir.ActivationFunctionType
    OP = mybir.AluOpType

    nc = bacc.Bacc(None, target_bir_lowering=False, debug=False,
                   num_devices=NCORE, num_swdge_queues=NQ)

    x_in = nc.dram_tensor("xb", [NPAD, C], bf16, kind="ExternalInput")
    idx_in = nc.dram_tensor("idx", [P, NTILE * 8], i16, kind="ExternalInput")
    colrel_in = nc.dram_tensor("colrel", [P, NMM], bf16, kind="ExternalInput")
    deg_in = nc.dram_tensor("dega", [P, NBLK], f32, kind="ExternalInput")
    dego_in = nc.dram_tensor("dego", [P, NB], f32, kind="ExternalInput")
    wgcnT_in = nc.dram_tensor("wgcnT", [P, CO, C], bf16, kind="ExternalInput")
    w1T_in = nc.dram_tensor("w1T", [P, CO, C], bf16, kind="ExternalInput")
    w2col_in = nc.dram_tensor("w2col", [P, CO, 1], bf16, kind="ExternalInput")
    bgcn_in = nc.dram_tensor("bgcn", [P, CO], f32, kind="ExternalInput")
    b1_in = nc.dram_tensor("b1", [P, CO], f32, kind="ExternalInput")
    b2_in = nc.dram_tensor("b2t", [P, 1], f32, kind="ExternalInput")
    iota_in = nc.dram_tensor("iota16", [P, BCO * P], bf16, kind="ExternalInput")
    ident_in = nc.dram_tensor("ident", [P, P], bf16, kind="ExternalInput")

    z_out = nc.dram_tensor("z", [P, NB], f32, kind="ExternalOutput")
    xs_q = [nc.dram_tensor(f"xs{q}", [SUBROWS, C], fp8) for q in range(SUB)]

    with ctile.TileContext(nc) as tc:
        with tc.tile_pool(name="const", bufs=1) as CPool:
            colrel_sb = CPool.tile([P, NMM], bf16)
            nc.sync.dma_start(colrel_sb[:], colrel_in[:])
            iota_sb = CPool.tile([P, BCO, P], bf16)
            nc.sync.dma_start(iota_sb[:],
                              iota_in[:].rearrange("p (j f) -> p j f", f=P))
            ident_sb = CPool.tile([P, P], bf16)
            nc.sync.dma_start(ident_sb[:], ident_in[:])
            wgcnT_sb = CPool.tile([P, CO, C], bf16)
            nc.sync.dma_start(wgcnT_sb[:], wgcnT_in[:])
            w1T_sb = CPool.tile([P, CO, C], bf16)
            nc.sync.dma_start(w1T_sb[:], w1T_in[:])
            w2col_sb = CPool.tile([P, CO, 1], bf16)
            nc.sync.dma_start(w2col_sb[:], w2col_in[:])
            bgcn_sb = CPool.tile([P, CO], f32)
            nc.sync.dma_start(bgcn_sb[:], bgcn_in[:])
            b1_sb = CPool.tile([P, CO], f32)
            nc.sync.dma_start(b1_sb[:], b1_in[:])
            b2_sb = CPool.tile([P, 1], f32)
            nc.sync.dma_start(b2_sb[:], b2_in[:])
            z_sb = CPool.tile([P, NB], f32)

            # dinv = rsqrt(deg): global (xs build) and own-blocks (drain)
            dega_sb = CPool.tile([P, NBLK], f32)
            nc.sync.dma_start(dega_sb[:], deg_in[:])
            dinv_sb = CPool.tile([P, NBLK], f32)
            nc.scalar.activation(dinv_sb[:], dega_sb[:], AF.Sqrt)
            nc.vector.reciprocal(dinv_sb[:], dinv_sb[:])
            dinva_bf = CPool.tile([P, NBLK], bf16)
            nc.vector.tensor_copy(dinva_bf[:], dinv_sb[:])
            dego_sb = CPool.tile([P, NB], f32)
            nc.sync.dma_start(dego_sb[:], dego_in[:])
            dinvo_sb = CPool.tile([P, NB], f32)
            nc.scalar.activation(dinvo_sb[:], dego_sb[:], AF.Sqrt)
            nc.vector.reciprocal(dinvo_sb[:], dinvo_sb[:])

            # ------- xs = dinv * x (fp8), per sub-table; ACT-engine DMAs ----
            # GpSimd helps only on sub 0 (it must finish before gathering
            # can start anyway); DVE alone finishes subs 1-3 while sub-0
            # gathers already run
            with tc.tile_pool(name="xsp", bufs=6) as xsp:
                it = 0
                for qq in range(SUB):
                    for b0 in range(0, SUBBLK, XB):
                        gb0 = qq * SUBBLK + b0
                        xt = xsp.tile([P, XB, C], bf16, tag="xt")
                        nc.scalar.dma_start(
                            xt[:],
                            x_in[gb0 * P:(gb0 + XB) * P, :]
                            .rearrange("(j p) c -> p j c", p=P))
                        xf = xsp.tile([P, XB, C], fp8, tag="xf")
                        eng = nc.gpsimd if (qq == 0 and it % 2 == 1) else nc.vector
                        eng.tensor_tensor(
                            xf[:], xt[:],
                            dinva_bf[:, gb0:gb0 + XB, None]
                            .to_broadcast([P, XB, C]),
                            OP.mult)
                        nc.scalar.dma_start(
                            xs_q[qq][b0 * P:(b0 + XB) * P, :]
                            .rearrange("(j p) c -> p j c", p=P),
                            xf[:])
                        it += 1

            with tc.tile_pool(name="idxp", bufs=3) as idxp, \
                 tc.tile_pool(name="gbp", bufs=6) as gbp, \
                 tc.tile_pool(name="m16p", bufs=4) as m16p, \
                 tc.tile_pool(name="ybfp", bufs=2) as ybfp, \
                 tc.tile_pool(name="yTp", bufs=2) as yTp, \
                 tc.tile_pool(name="gTp", bufs=2) as gTp, \
                 tc.tile_pool(name="hTp", bufs=2) as hTp, \
                 tc.tile_pool(name="zrp", bufs=2) as zrp, \
                 tc.tile_pool(name="yps", bufs=GRP, space="PSUM") as ypsp, \
                 tc.tile_pool(name="tpp", bufs=1, space="PSUM") as tpp, \
                 tc.tile_pool(name="mmp", bufs=1, space="PSUM") as mmp:

                qc = 0
                pending_mlp = None

                def emit_mlp(gg, gsz, yT):
                    b0blk = sum(GRP_SIZES[:gg])
                    chunks = [(0, min(4, gsz))] + ([(4, gsz)] if gsz > 4 else [])
                    gT = gTp.tile([P, CO, GRP, P], bf16, tag="gT")
                    hT = hTp.tile([P, CO, GRP, P], bf16, tag="hT")
                    for src, dst, wsb, bsb in ((yT, gT, wgcnT_sb, bgcn_sb),
                                               (gT, hT, w1T_sb, b1_sb)):
                        for oi in range(CO):
                            for j0, j1 in chunks:
                                mm = mmp.tile([P, 4, P], f32, tag="mm")
                                cb = j1 - j0
                                for ci in range(CO):
                                    if src is yT:
                                        rhs = src[:, j0:j1, ci, :]
                                    else:
                                        rhs = src[:, ci, j0:j1, :]
                                    nc.tensor.matmul(
                                        mm[:, :cb, :],
                                        lhsT=wsb[:, ci, oi * P:(oi + 1) * P],
                                        rhs=rhs,
                                        start=(ci == 0), stop=(ci == CO - 1))
                                nc.scalar.activation(
                                    dst[:, oi, j0:j1, :], mm[:, :cb, :],
                                    AF.Relu, bias=bsb[:, oi:oi + 1])
                    zp = mmp.tile([P, GRP], f32, tag="mm")
                    for j in range(gsz):
                        for ci in range(CO):
                            nc.tensor.matmul(
                                zp[:, j:j + 1], lhsT=hT[:, ci, j, :],
                                rhs=w2col_sb[:, ci, :],
                                start=(ci == 0), stop=(ci == CO - 1))
                    zr = zrp.tile([P, GRP], f32, tag="zr")
                    nc.vector.tensor_scalar(zr[:, :gsz], zp[:, :gsz],
                                            b2_sb[:], 0.0, OP.add, OP.max)
                    nc.scalar.activation(z_sb[:, b0blk:b0blk + gsz],
                                         zr[:, :gsz], AF.Sigmoid)

                for gg in range(NGRP):
                    gsz = GRP_SIZES[gg]
                    b0blk = sum(GRP_SIZES[:gg])
                    tg0 = int(toff[gg * SUB])
                    tgn = int(toff[(gg + 1) * SUB]) - tg0
                    ib = idxp.tile([P, TGMAX * 8], i16, tag="ib")
                    nc.sync.dma_start(ib[:, :tgn * 8],
                                      idx_in[:, tg0 * 8:(tg0 + tgn) * 8])
                    gbs = []
                    for qq in range(SUB):
                        gqi = gg * SUB + qq
                        tgq = int(T[gqi])
                        tb0 = int(toff[gqi]) - tg0      # tile base within ib
                        gb = gbp.tile([P, TGQMAX, C], fp8, tag="gb")
                        for c0 in range(0, tgq, GMAX):
                            cn = min(GMAX, tgq - c0)
                            nc.gpsimd.dma_gather(
                                gb[:, c0:c0 + cn, :],
                                xs_q[qq][:],
                                ib[:, (tb0 + c0) * 8:(tb0 + c0 + cn) * 8],
                                num_idxs=cn * P,
                                num_idxs_reg=cn * P,
                                elem_size=C,
                                elem_step=C,
                                queue_num=qc % NQ,
                            )
                            qc += 1
                        gbs.append(gb)

                    ypt = [ypsp.tile([P, C], f32, tag="y", name=f"yp{j}")
                           for j in range(gsz)]
                    mbase, mcount = grp_nmm[gg]
                    for m0 in range(mbase, mbase + mcount, BCO):
                        m16 = m16p.tile([P, BCO, P], fp8, tag="c16")
                        nc.vector.tensor_tensor(
                            m16[:],
                            colrel_sb[:, m0:m0 + BCO, None].to_broadcast(
                                [P, BCO, P]),
                            iota_sb[:], OP.is_equal)
                        for mi in range(m0, m0 + BCO):
                            if e_gq[mi] < 0:
                                continue        # pad entry
                            qq = int(e_gq[mi]) - gg * SUB
                            nc.tensor.matmul(
                                ypt[int(e_s[mi])][:],
                                lhsT=m16[:, mi - m0, :],
                                rhs=gbs[qq][:, int(e_t[mi]), :],
                                start=bool(e_start[mi]),
                                stop=bool(e_stop[mi]))

                    # drain y (ACT: cast + dinv[tgt] scale)
                    ybf = ybfp.tile([P, GRP, C], bf16, tag="ybf")
                    for j in range(gsz):
                        nc.scalar.activation(
                            ybf[:, j, :], ypt[j][:], AF.Copy,
                            scale=dinvo_sb[:, b0blk + j:b0blk + j + 1])

                    # MLP of the previous grp overlaps this grp's transposes
                    if pending_mlp is not None:
                        emit_mlp(*pending_mlp)

                    # transpose y via TensorE (no DMA: keeps HWDGE lanes free)
                    yT = yTp.tile([P, GRP, CO, P], bf16, tag="yT")
                    for j in range(gsz):
                        for ci in range(CO):
                            tp = tpp.tile([P, P], bf16, tag="tp")
                            nc.tensor.transpose(
                                tp[:], ybf[:, j, ci * P:(ci + 1) * P],
                                ident_sb[:])
                            nc.scalar.activation(yT[:, j, ci, :], tp[:],
                                                 AF.Copy)
                    pending_mlp = (gg, gsz, yT)

                emit_mlp(*pending_mlp)
                nc.sync.dma_start(z_out[:], z_sb[:])

    nc.compile()
    return nc


# ----------------------------------------------------------------------------
# entry point
# ----------------------------------------------------------------------------

def _install_ntff_hook():
    """Best-effort: register the axon NTFF profile hook so trace=True works."""
    import sys, types, contextlib, ctypes
    if "antenv.axon_hooks" in sys.modules:
        return True
    try:
        lib = ctypes.CDLL("/opt/axon/libaxon_pjrt.so")
        if not hasattr(lib, "axon_start_nrt_profile"):
            return False
        lib.axon_start_nrt_profile.argtypes = [ctypes.POINTER(ctypes.c_int64), ctypes.c_size_t]
        lib.axon_start_nrt_profile.restype = ctypes.c_int64
        lib.axon_stop_nrt_profile.argtypes = [ctypes.c_char_p]
        lib.axon_stop_nrt_profile.restype = ctypes.c_int64

        @contextlib.contextmanager
        def _hook(output_dir, device_ids):
            import jax
            jax.devices()
            if device_ids:
                ids = (ctypes.c_int64 * len(device_ids))(*device_ids)
                rc = lib.axon_start_nrt_profile(ids, len(device_ids))
            else:
                rc = lib.axon_start_nrt_profile(None, 0)
            if rc != 0:
                raise RuntimeError(f"axon_start_nrt_profile rc={rc}")
            try:
                yield
            finally:
                n = lib.axon_stop_nrt_profile(str(output_dir).encode())
                if n < 0:
                    raise RuntimeError(f"axon_stop_nrt_profile rc={n}")

        mod = types.ModuleType("antenv.axon_hooks")
        mod.get_axon_ntff_profile_hook = lambda: _hook
        mod.set_axon_ntff_profile_hook = lambda h: None
        sys.modules["antenv.axon_hooks"] = mod
        return True
    except Exception:
        return False


def kernel(x, edge_index, W_gcn, b_gcn, W1, b1, W2, b2, _trace=None):
    global LAST_EXEC_NS
    from concourse.bass_utils import run_bass_kernel_spmd

    x = np.asarray(x, dtype=np.float32)
    edge_index = np.asarray(edge_index)
    sched, idx_in, colrel_in, deg_tbl, deg_own = _preprocess(edge_index)
    wd = _prep_weights(W_gcn, b_gcn, W1, b1, W2, b2)

    x_pad = np.zeros((NPAD, C), dtype=_BF16)
    x_pad[:N] = x.astype(_BF16)

    nc = _build(sched)
    in_maps = []
    for k in range(NCORE):
        in_maps.append(dict(
            xb=x_pad,
            idx=np.ascontiguousarray(idx_in[k]),
            colrel=np.ascontiguousarray(colrel_in[k]),
            dega=deg_tbl,
            dego=np.ascontiguousarray(deg_own[k]),
            wgcnT=wd["wgcnT"], w1T=wd["w1T"], w2col=wd["w2col"],
            bgcn=wd["bgcn"], b1=wd["b1"],
            b2t=np.full((P, 1), wd["b2"], dtype=np.float32),
            iota16=wd["iota16"], ident=wd["ident"],
        ))

    trace = _trace if _trace is not None else _install_ntff_hook()
    res = run_bass_kernel_spmd(nc, in_maps, core_ids=list(range(NCORE)),
                               trace=bool(trace))
    LAST_EXEC_NS = res.exec_time_ns

    out = np.zeros((NBLK * P,), np.float32)
    own_glb = sched["own_glb"]
    for k in range(NCORE):
        zk = np.asarray(res.results[k]["z"]).astype(np.float32)  # [128, NB]
        # core k's local block i holds global block own_glb[k, i]
        out.reshape(NBLK, P)[own_glb[k]] = zk.T
    return out[:N].reshape(N, 1)


# revision 32
# speedup vs baseline: 2.1317x; 1.2054x over previous
"""GCN message-passing kernel for 8 TRN2 NeuronCores (Bass/Tile), v4.

Math (equivalent to the PyG-style reference):
    deg[i]  = 1 + #{edges with target i}              (self-loops added)
    dinv    = deg^-1/2
    y[i]    = dinv[i] * sum_{j -> i} dinv[j] * x[j]   (incl. self loop j=i)
    g       = relu(y @ Wg^T + bg)
    h       = relu(g @ W1^T + b1)
    out     = sigmoid(relu(h @ W2^T + b2))

Design notes:
  - Host does index work only: shard/sort/pad edges, count degrees.
    Device computes dinv = rsqrt(deg), the scaled table xs = dinv*x (fp8),
    all matmuls and activations.
  - Nodes sharded: core k owns 98 blocks of 128 targets, grouped into 17
    grps of <=6 (one PSUM bank per block accumulator; 2 banks left for the
    MLP and TensorE transposes).
  - Edges sorted by (grp, src sub-table, target block); each (grp, sub,
    block) segment is padded to a cross-core common length so one SPMD
    program serves all cores (colrel sentinels mask absent rows).
  - dma_gather pulls xs rows in 1024-idx calls cycled over 4 SWDGE queues
    (the Q7 descriptor-generation rate, ~2.8ns/idx with 4 queues, is the
    kernel's critical resource; single-queue is 3x slower).
  - Steady-state HWDGE traffic is ONE idx load per grp: everything else
    (transposes, drains, MLP) stays on-chip, because HWDGE completion
    semaphore lanes are shared across engines and queued DMAs would
    false-serialize the gather stream.
  - Aggregation: per 128-edge tile, a pure one-hot fp8 matrix (one DVE
    is_equal from a colrel table) scatters gathered rows into the block's
    PSUM via TensorE matmul; dinv[tgt] rides the ACT drain (Copy*scale).
"""

import math

import numpy as np
import ml_dtypes

P = 128
C = 256
CO = 2                      # C // P
NCORE = 8
N = 100000
NB = 98                     # blocks per core
NBLK = NB * NCORE           # 784
NPAD = NBLK * P             # 100352
GRP = 6                     # max blocks per psum group
GRP_SIZES = [6] * 16 + [2]  # 17 grps covering 98 blocks
NGRP = len(GRP_SIZES)
SUB = 4                     # gather sub-tables (int16 index limit)
SUBROWS = NPAD // SUB       # 25088
SUBBLK = SUBROWS // P       # 196
NGQ = NGRP * SUB
GMAX = 8                    # tiles per dma_gather call (1024-idx ring limit)
NQ = 4                      # SWDGE queues
BCO = 16                    # coeff tiles built per DVE instruction
XB = 7                      # x blocks per xs-build step

_BF16 = ml_dtypes.bfloat16
_FP8 = ml_dtypes.float8_e4m3

LAST_EXEC_NS = None


# ----------------------------------------------------------------------------
# host-side preprocessing (index work: shard, sort, pad, count)
# ----------------------------------------------------------------------------

def _preprocess(edge_index):
    row = np.ascontiguousarray(edge_index[0]).astype(np.int64)
    col = np.ascontiguousarray(edge_index[1]).astype(np.int64)
    loop = np.arange(N, dtype=np.int64)
    row = np.concatenate([row, loop])
    col = np.concatenate([col, loop])

    deg = np.bincount(col, minlength=NPAD).astype(np.float32)
    deg[N:] = 1.0
    deg_tbl = np.ascontiguousarray(deg.reshape(NBLK, P).T)      # [P, NBLK]

    # assign target blocks to (core, local slot) grouped by (self-loop
    # sub-table, degree rank): the 8 blocks sharing a (grp, slot, sub)
    # segment then have the same self-sub and near-equal edge counts, which
    # minimizes the pad-to-max-over-cores cost and balances total load
    blkdeg = deg.reshape(NBLK, P).sum(axis=1)
    sub_of_blk = np.arange(NBLK) // SUBBLK
    rank_order = np.lexsort((-blkdeg, sub_of_blk))              # rank -> glb
    perm_core = np.empty(NBLK, np.int64)
    perm_loc = np.empty(NBLK, np.int64)
    perm_core[rank_order] = np.arange(NBLK) % NCORE
    perm_loc[rank_order] = np.arange(NBLK) // NCORE

    blk = col >> 7
    core = perm_core[blk]
    bl = perm_loc[blk]
    g = bl // GRP
    s = bl % GRP
    q = row // SUBROWS
    gq = g * SUB + q

    order = np.lexsort((col, s, gq, core))
    row_s = row[order]
    col_s = col[order]
    core_s = core[order]
    q_s = q[order]

    # segment = (core, gq, slot); common padded length per (gq, slot)
    gqs = (gq * GRP + s)
    seg = core * (NGQ * GRP) + gqs
    seg_s = seg[order]
    cnt3 = np.bincount(seg, minlength=NCORE * NGQ * GRP).reshape(NCORE, -1)
    S = cnt3.max(axis=0).reshape(NGQ, GRP)                      # [NGQ, GRP]
    soff = np.zeros((NGQ, GRP + 1), np.int64)
    np.cumsum(S, axis=1, out=soff[:, 1:])
    L = soff[:, -1]                                             # edges per gq
    T = (L + P - 1) // P                                        # tiles per gq
    toff = np.zeros(NGQ + 1, np.int64)
    np.cumsum(T, out=toff[1:])
    NTILE = int(toff[-1])
    NIDX = NTILE * P

    # entry schedule: (gq, tile, slots overlapping) in order; BCO-pad per grp
    Tmax = int(T.max())
    lut = np.full((NGQ, Tmax, GRP), -1, np.int64)
    e_gq, e_t, e_s = [], [], []
    e_start, e_stop = [], []
    seen_start = set()
    grp_nmm = []
    m = 0
    for gg in range(NGRP):
        m0 = m
        nslot = GRP_SIZES[gg]
        for qq in range(SUB):
            gqi = gg * SUB + qq
            t_lo = soff[gqi, :-1] // P
            t_hi = (soff[gqi, 1:] - 1) // P + 1
            for t in range(int(T[gqi])):
                for ss in range(nslot):
                    if S[gqi, ss] > 0 and t_lo[ss] <= t < t_hi[ss]:
                        lut[gqi, t, ss] = m
                        e_gq.append(gqi); e_t.append(t); e_s.append(ss)
                        key = (gg, ss)
                        e_start.append(key not in seen_start)
                        seen_start.add(key)
                        e_stop.append(False)
                        m += 1
        seen = set()
        for i in range(m - 1, m0 - 1, -1):
            if e_s[i] not in seen:
                e_stop[i] = True
                seen.add(e_s[i])
                if len(seen) == nslot:
                    break
        npad = (-(m - m0)) % BCO
        for _ in range(npad):
            e_gq.append(-1); e_t.append(0); e_s.append(0)
            e_start.append(False); e_stop.append(False)
        m += npad
        grp_nmm.append((m0, m - m0))
    NMM = m
    e_gq = np.array(e_gq); e_t = np.array(e_t); e_s = np.array(e_s)
    e_start = np.array(e_start); e_stop = np.array(e_stop)

    # per-core data tables
    idx_dat = np.zeros((NCORE, NIDX), np.int16)
    colrel_dat = np.full((NCORE, NMM * P), 254.0, np.float32)

    seg_start = np.zeros(NCORE * NGQ * GRP + 1, np.int64)
    np.cumsum(cnt3.reshape(-1), out=seg_start[1:])
    pos_in_seg = np.arange(len(row_s), dtype=np.int64) - seg_start[seg_s]
    gqs_s = gqs[order]
    pos_in_gq = soff[:, :-1].reshape(-1)[gqs_s] + pos_in_seg    # within gq
    tile_e = pos_in_gq // P
    p_e = pos_in_gq % P
    gpos = toff[gqs_s // GRP] * P + pos_in_gq
    m_e = lut[gqs_s // GRP, tile_e, gqs_s % GRP]
    assert (m_e >= 0).all()
    idx_val = (row_s - q_s * SUBROWS).astype(np.int16)
    colv = (col_s & 127).astype(np.float32)   # within-block target, unpermuted
    flat_m = m_e * P + p_e
    for k in range(NCORE):
        sel = core_s == k
        idx_dat[k, gpos[sel]] = idx_val[sel]
        colrel_dat[k, flat_m[sel]] = colv[sel]

    idx_in = np.ascontiguousarray(
        np.tile(idx_dat.reshape(NCORE, NIDX // 16, 16).transpose(0, 2, 1),
                (1, 8, 1)))                                 # [NCORE,128,NIDX//16]
    colrel_in = np.ascontiguousarray(
        colrel_dat.reshape(NCORE, NMM, P).transpose(0, 2, 1)).astype(_BF16)

    own_glb = np.stack([rank_order[np.arange(NB) * NCORE + k]
                        for k in range(NCORE)])                  # [NCORE, NB]
    deg_own = np.stack([deg_tbl[:, own_glb[k]] for k in range(NCORE)])

    sched = dict(T=T, toff=toff, NTILE=NTILE, NMM=NMM, grp_nmm=grp_nmm,
                 e_gq=e_gq, e_t=e_t, e_s=e_s, e_start=e_start, e_stop=e_stop,
                 own_glb=own_glb)
    return sched, idx_in, colrel_in, deg_tbl, np.ascontiguousarray(deg_own)


def _prep_weights(W_gcn, b_gcn, W1, b1, W2, b2):
    def wT(W):  # [C,C] -> lhsT layout [128, CO, C]: [p, ci, o] = W[o, ci*128+p]
        return np.ascontiguousarray(
            np.asarray(W).T.reshape(CO, P, C).transpose(1, 0, 2)).astype(_BF16)
    w2col = np.ascontiguousarray(
        np.asarray(W2).reshape(C).reshape(CO, P).transpose(1, 0)[:, :, None]
    ).astype(_BF16)
    bg = np.ascontiguousarray(np.asarray(b_gcn).reshape(CO, P).T).astype(np.float32)
    bb1 = np.ascontiguousarray(np.asarray(b1).reshape(CO, P).T).astype(np.float32)
    iota16 = np.tile(np.arange(P, dtype=np.float32), (P, BCO)).astype(_BF16)
    ident = np.eye(P, dtype=np.float32).astype(_BF16)
    return dict(wgcnT=wT(W_gcn), w1T=wT(W1), w2col=w2col, bgcn=bg, b1=bb1,
                b2=float(np.asarray(b2).reshape(-1)[0]), iota16=iota16,
                ident=ident)


# ----------------------------------------------------------------------------
# device program (SPMD: one program, 8 cores; per-core data differs)
# ----------------------------------------------------------------------------

def _build(sched):
    from concourse import bacc, mybir
    from concourse import tile as ctile

    T = sched["T"]
    toff = sched["toff"]
    NTILE = sched["NTILE"]
    NMM = sched["NMM"]
    grp_nmm = sched["grp_nmm"]
    e_gq = sched["e_gq"]
    e_t = sched["e_t"]
    e_s = sched["e_s"]
    e_start = sched["e_start"]
    e_stop = sched["e_stop"]
    TGQMAX = int(T.max())
    TGMAX = int(max(toff[(gg + 1) * SUB] - toff[gg * SUB]
                    for gg in range(NGRP)))

    f32 = mybir.dt.float32
    bf16 = mybir.dt.bfloat16
    fp8 = mybir.dt.float8e4
    i16 = mybir.dt.int16
    AF = mybir.ActivationFunctionType
    OP = mybir.AluOpType

    nc = bacc.Bacc(None, target_bir_lowering=False, debug=False,
                   num_devices=NCORE, num_swdge_queues=NQ)

    x_in = nc.dram_tensor("xb", [NPAD, C], bf16, kind="ExternalInput")
    idx_in = nc.dram_tensor("idx", [P, NTILE * 8], i16, kind="ExternalInput")
    colrel_in = nc.dram_tensor("colrel", [P, NMM], bf16, kind="ExternalInput")
    deg_in = nc.dram_tensor("dega", [P, NBLK], f32, kind="ExternalInput")
    dego_in = nc.dram_tensor("dego", [P, NB], f32, kind="ExternalInput")
    wgcnT_in = nc.dram_tensor("wgcnT", [P, CO, C], bf16, kind="ExternalInput")
    w1T_in = nc.dram_tensor("w1T", [P, CO, C], bf16, kind="ExternalInput")
    w2col_in = nc.dram_tensor("w2col", [P, CO, 1], bf16, kind="ExternalInput")
    bgcn_in = nc.dram_tensor("bgcn", [P, CO], f32, kind="ExternalInput")
    b1_in = nc.dram_tensor("b1", [P, CO], f32, kind="ExternalInput")
    b2_in = nc.dram_tensor("b2t", [P, 1], f32, kind="ExternalInput")
    iota_in = nc.dram_tensor("iota16", [P, BCO * P], bf16, kind="ExternalInput")
    ident_in = nc.dram_tensor("ident", [P, P], bf16, kind="ExternalInput")

    z_out = nc.dram_tensor("z", [P, NB], f32, kind="ExternalOutput")
    xs_q = [nc.dram_tensor(f"xs{q}", [SUBROWS, C], fp8) for q in range(SUB)]

    with ctile.TileContext(nc) as tc:
        with tc.tile_pool(name="const", bufs=1) as CPool:
            colrel_sb = CPool.tile([P, NMM], bf16)
            nc.sync.dma_start(colrel_sb[:], colrel_in[:])
            iota_sb = CPool.tile([P, BCO, P], bf16)
            nc.sync.dma_start(iota_sb[:],
                              iota_in[:].rearrange("p (j f) -> p j f", f=P))
            ident_sb = CPool.tile([P, P], bf16)
            nc.sync.dma_start(ident_sb[:], ident_in[:])
            wgcnT_sb = CPool.tile([P, CO, C], bf16)
            nc.sync.dma_start(wgcnT_sb[:], wgcnT_in[:])
            w1T_sb = CPool.tile([P, CO, C], bf16)
            nc.sync.dma_start(w1T_sb[:], w1T_in[:])
            w2col_sb = CPool.tile([P, CO, 1], bf16)
            nc.sync.dma_start(w2col_sb[:], w2col_in[:])
            bgcn_sb = CPool.tile([P, CO], f32)
            nc.sync.dma_start(bgcn_sb[:], bgcn_in[:])
            b1_sb = CPool.tile([P, CO], f32)
            nc.sync.dma_start(b1_sb[:], b1_in[:])
            b2_sb = CPool.tile([P, 1], f32)
            nc.sync.dma_start(b2_sb[:], b2_in[:])
            z_sb = CPool.tile([P, NB], f32)

            # dinv = rsqrt(deg): global (xs build) and own-blocks (drain)
            dega_sb = CPool.tile([P, NBLK], f32)
            nc.sync.dma_start(dega_sb[:], deg_in[:])
            dinv_sb = CPool.tile([P, NBLK], f32)
            nc.scalar.activation(dinv_sb[:], dega_sb[:], AF.Sqrt)
            nc.vector.reciprocal(dinv_sb[:], dinv_sb[:])
            dinva_bf = CPool.tile([P, NBLK], bf16)
            nc.vector.tensor_copy(dinva_bf[:], dinv_sb[:])
            dego_sb = CPool.tile([P, NB], f32)
            nc.sync.dma_start(dego_sb[:], dego_in[:])
            dinvo_sb = CPool.tile([P, NB], f32)
            nc.scalar.activation(dinvo_sb[:], dego_sb[:], AF.Sqrt)
            nc.vector.reciprocal(dinvo_sb[:], dinvo_sb[:])

            with tc.tile_pool(name="xsp", bufs=6) as xsp, \
                 tc.tile_pool(name="idxp", bufs=4) as idxp, \
                 tc.tile_pool(name="gbp", bufs=6) as gbp, \
                 tc.tile_pool(name="m16p", bufs=4) as m16p, \
                 tc.tile_pool(name="ybfp", bufs=2) as ybfp, \
                 tc.tile_pool(name="yTp", bufs=2) as yTp, \
                 tc.tile_pool(name="gTp", bufs=2) as gTp, \
                 tc.tile_pool(name="hTp", bufs=2) as hTp, \
                 tc.tile_pool(name="zrp", bufs=2) as zrp, \
                 tc.tile_pool(name="yps", bufs=GRP, space="PSUM") as ypsp, \
                 tc.tile_pool(name="tpp", bufs=1, space="PSUM") as tpp, \
                 tc.tile_pool(name="mmp", bufs=1, space="PSUM") as mmp:

                def load_ib(gg):
                    tg0 = int(toff[gg * SUB])
                    tgn = int(toff[(gg + 1) * SUB]) - tg0
                    ib = idxp.tile([P, TGMAX * 8], i16, tag="ib",
                                   name=f"ib{gg}")
                    nc.sync.dma_start(ib[:, :tgn * 8],
                                      idx_in[:, tg0 * 8:(tg0 + tgn) * 8])
                    return ib

                # idx tiles for the first grps load ahead of the xs reads
                ibs = {gg: load_ib(gg) for gg in range(4)}

                # ---- xs = dinv * x (fp8), per sub-table ----
                # reads on sync DGE, writes on scalar DGE (no same-FIFO
                # head-of-line between write(i) and read(i+1)); mults on
                # DVE only - GpSimd must run nothing but dma_gather, any
                # other ucode forces a library reload that quiesces ALL
                # in-flight DMA
                for qq in range(SUB):
                    for b0 in range(0, SUBBLK, XB):
                        gb0 = qq * SUBBLK + b0
                        xt = xsp.tile([P, XB, C], bf16, tag="xt")
                        nc.sync.dma_start(
                            xt[:],
                            x_in[gb0 * P:(gb0 + XB) * P, :]
                            .rearrange("(j p) c -> p j c", p=P))
                        xf = xsp.tile([P, XB, C], fp8, tag="xf")
                        nc.vector.tensor_tensor(
                            xf[:], xt[:],
                            dinva_bf[:, gb0:gb0 + XB, None]
                            .to_broadcast([P, XB, C]),
                            OP.mult)
                        nc.scalar.dma_start(
                            xs_q[qq][b0 * P:(b0 + XB) * P, :]
                            .rearrange("(j p) c -> p j c", p=P),
                            xf[:])

                qc = 0
                pending_mlp = None

                def emit_mlp(gg, gsz, yT):
                    b0blk = sum(GRP_SIZES[:gg])
                    chunks = [(0, min(4, gsz))] + ([(4, gsz)] if gsz > 4 else [])
                    gT = gTp.tile([P, CO, GRP, P], bf16, tag="gT")
                    hT = hTp.tile([P, CO, GRP, P], bf16, tag="hT")
                    for src, dst, wsb, bsb in ((yT, gT, wgcnT_sb, bgcn_sb),
                                               (gT, hT, w1T_sb, b1_sb)):
                        for oi in range(CO):
                            for j0, j1 in chunks:
                                mm = mmp.tile([P, 4, P], f32, tag="mm")
                                cb = j1 - j0
                                for ci in range(CO):
                                    if src is yT:
                                        rhs = src[:, j0:j1, ci, :]
                                    else:
                                        rhs = src[:, ci, j0:j1, :]
                                    nc.tensor.matmul(
                                        mm[:, :cb, :],
                                        lhsT=wsb[:, ci, oi * P:(oi + 1) * P],
                                        rhs=rhs,
                                        start=(ci == 0), stop=(ci == CO - 1))
                                nc.scalar.activation(
                                    dst[:, oi, j0:j1, :], mm[:, :cb, :],
                                    AF.Relu, bias=bsb[:, oi:oi + 1])
                    zp = mmp.tile([P, GRP], f32, tag="mm")
                    for j in range(gsz):
                        for ci in range(CO):
                            nc.tensor.matmul(
                                zp[:, j:j + 1], lhsT=hT[:, ci, j, :],
                                rhs=w2col_sb[:, ci, :],
                                start=(ci == 0), stop=(ci == CO - 1))
                    zr = zrp.tile([P, GRP], f32, tag="zr")
                    nc.vector.tensor_scalar(zr[:, :gsz], zp[:, :gsz],
                                            b2_sb[:], 0.0, OP.add, OP.max)
                    nc.scalar.activation(z_sb[:, b0blk:b0blk + gsz],
                                         zr[:, :gsz], AF.Sigmoid)

                for gg in range(NGRP):
                    gsz = GRP_SIZES[gg]
                    b0blk = sum(GRP_SIZES[:gg])
                    tg0 = int(toff[gg * SUB])
                    ib = ibs.pop(gg) if gg in ibs else load_ib(gg)
                    gbs = []
                    for qq in range(SUB):
                        gqi = gg * SUB + qq
                        tgq = int(T[gqi])
                        tb0 = int(toff[gqi]) - tg0      # tile base within ib
                        gb = gbp.tile([P, TGQMAX, C], fp8, tag="gb")
                        for c0 in range(0, tgq, GMAX):
                            cn = min(GMAX, tgq - c0)
                            nc.gpsimd.dma_gather(
                                gb[:, c0:c0 + cn, :],
                                xs_q[qq][:],
                                ib[:, (tb0 + c0) * 8:(tb0 + c0 + cn) * 8],
                                num_idxs=cn * P,
                                num_idxs_reg=cn * P,
                                elem_size=C,
                                elem_step=C,
                                queue_num=qc % NQ,
                            )
                            qc += 1
                        gbs.append(gb)

                    ypt = [ypsp.tile([P, C], f32, tag="y", name=f"yp{j}")
                           for j in range(gsz)]
                    mbase, mcount = grp_nmm[gg]
                    for m0 in range(mbase, mbase + mcount, BCO):
                        m16 = m16p.tile([P, BCO, P], fp8, tag="c16")
                        nc.vector.tensor_tensor(
                            m16[:],
                            colrel_sb[:, m0:m0 + BCO, None].to_broadcast(
                                [P, BCO, P]),
                            iota_sb[:], OP.is_equal)
                        for mi in range(m0, m0 + BCO):
                            if e_gq[mi] < 0:
                                continue        # pad entry
                            qq = int(e_gq[mi]) - gg * SUB
                            nc.tensor.matmul(
                                ypt[int(e_s[mi])][:],
                                lhsT=m16[:, mi - m0, :],
                                rhs=gbs[qq][:, int(e_t[mi]), :],
                                start=bool(e_start[mi]),
                                stop=bool(e_stop[mi]))

                    # drain y (ACT: cast + dinv[tgt] scale)
                    ybf = ybfp.tile([P, GRP, C], bf16, tag="ybf")
                    for j in range(gsz):
                        nc.scalar.activation(
                            ybf[:, j, :], ypt[j][:], AF.Copy,
                            scale=dinvo_sb[:, b0blk + j:b0blk + j + 1])

                    # MLP of the previous grp overlaps this grp's transposes
                    if pending_mlp is not None:
                        emit_mlp(*pending_mlp)

                    # transpose y via TensorE (no DMA: keeps HWDGE lanes free)
                    yT = yTp.tile([P, GRP, CO, P], bf16, tag="yT")
                    for j in range(gsz):
                        for ci in range(CO):
                            tp = tpp.tile([P, P], bf16, tag="tp")
                            nc.tensor.transpose(
                                tp[:], ybf[:, j, ci * P:(ci + 1) * P],
                                ident_sb[:])
                            nc.scalar.activation(yT[:, j, ci, :], tp[:],
                                                 AF.Copy)
                    pending_mlp = (gg, gsz, yT)

                emit_mlp(*pending_mlp)
                nc.sync.dma_start(z_out[:], z_sb[:])

    nc.compile()
    return nc


# ----------------------------------------------------------------------------
# entry point
# ----------------------------------------------------------------------------

def _install_ntff_hook():
    """Best-effort: register the axon NTFF profile hook so trace=True works."""
    import sys, types, contextlib, ctypes
    if "antenv.axon_hooks" in sys.modules:
        return True
    try:
        lib = ctypes.CDLL("/opt/axon/libaxon_pjrt.so")
        if not hasattr(lib, "axon_start_nrt_profile"):
            return False
        lib.axon_start_nrt_profile.argtypes = [ctypes.POINTER(ctypes.c_int64), ctypes.c_size_t]
        lib.axon_start_nrt_profile.restype = ctypes.c_int64
        lib.axon_stop_nrt_profile.argtypes = [ctypes.c_char_p]
        lib.axon_stop_nrt_profile.restype = ctypes.c_int64

        @contextlib.contextmanager
        def _hook(output_dir, device_ids):
            import jax
            jax.devices()
            if device_ids:
                ids = (ctypes.c_int64 * len(device_ids))(*device_ids)
                rc = lib.axon_start_nrt_profile(ids, len(device_ids))
            else:
                rc = lib.axon_start_nrt_profile(None, 0)
            if rc != 0:
                raise RuntimeError(f"axon_start_nrt_profile rc={rc}")
            try:
                yield
            finally:
                n = lib.axon_stop_nrt_profile(str(output_dir).encode())
                if n < 0:
                    raise RuntimeError(f"axon_stop_nrt_profile rc={n}")

        mod = types.ModuleType("antenv.axon_hooks")
        mod.get_axon_ntff_profile_hook = lambda: _hook
        mod.set_axon_ntff_profile_hook = lambda h: None
        sys.modules["antenv.axon_hooks"] = mod
        return True
    except Exception:
        return False


def kernel(x, edge_index, W_gcn, b_gcn, W1, b1, W2, b2, _trace=None):
    global LAST_EXEC_NS
    from concourse.bass_utils import run_bass_kernel_spmd

    x = np.asarray(x, dtype=np.float32)
    edge_index = np.asarray(edge_index)
    sched, idx_in, colrel_in, deg_tbl, deg_own = _preprocess(edge_index)
    wd = _prep_weights(W_gcn, b_gcn, W1, b1, W2, b2)

    x_pad = np.zeros((NPAD, C), dtype=_BF16)
    x_pad[:N] = x.astype(_BF16)

    nc = _build(sched)
    in_maps = []
    for k in range(NCORE):
        in_maps.append(dict(
            xb=x_pad,
            idx=np.ascontiguousarray(idx_in[k]),
            colrel=np.ascontiguousarray(colrel_in[k]),
            dega=deg_tbl,
            dego=np.ascontiguousarray(deg_own[k]),
            wgcnT=wd["wgcnT"], w1T=wd["w1T"], w2col=wd["w2col"],
            bgcn=wd["bgcn"], b1=wd["b1"],
            b2t=np.full((P, 1), wd["b2"], dtype=np.float32),
            iota16=wd["iota16"], ident=wd["ident"],
        ))

    trace = _trace if _trace is not None else _install_ntff_hook()
    res = run_bass_kernel_spmd(nc, in_maps, core_ids=list(range(NCORE)),
                               trace=bool(trace))
    LAST_EXEC_NS = res.exec_time_ns

    out = np.zeros((NBLK * P,), np.float32)
    own_glb = sched["own_glb"]
    for k in range(NCORE):
        zk = np.asarray(res.results[k]["z"]).astype(np.float32)  # [128, NB]
        # core k's local block i holds global block own_glb[k, i]
        out.reshape(NBLK, P)[own_glb[k]] = zk.T
    return out[:N].reshape(N, 1)
